# revision 1
# baseline (speedup 1.0000x reference)
"""Trainium2 Bass kernel for nn_DisentangledGraphConvEncoder (octo v4).

Sharding: core k owns dst eighth k (6272 padded rows) and computes ALL 8
channels for those dsts. Tables are [NPAD, 512] bf16 (8 ch x 64 feats = 1KB
rows) so each gather descriptor moves 1KB.

v4 refinements over v3:
- Per-core groups are permuted by descending edge count before slot
  assignment, so the SPMD-common (max-over-cores) chunk layout wastes ~12%
  instead of ~24%.
- The h1 table uses a half-major layout (half p of every eighth, rank-major)
  so the inter-layer AllGather splits into two contiguous-output collectives;
  part 1 overlaps the tail of layer a.
- Layer a gathers from h0 in node order; layer b gathers from h1 in
  slot/half-major order — separate idx16 + S streams per layer.
"""

import numpy as np
import ml_dtypes

import concourse.bass as bass
import concourse.bacc as bacc
import concourse.tile as tile
from concourse import mybir
from concourse import bass_utils

F32 = mybir.dt.float32
BF16 = mybir.dt.bfloat16
NPBF16 = ml_dtypes.bfloat16

N = 50000
E = 800000
D = 256
C = 8
H = 64
GRP = 64
NPAD = 50176            # 784 groups of 64; divisible by 8*64
QROWS = NPAD // 8       # 6272 rows per dst eighth
QGRP = QROWS // GRP     # 98 groups per eighth
HR = QROWS // 2         # 3136 rows: half of an eighth (cc split unit)
LO_ROWS = 25088         # = 8*HR: lo gather-table rows (mult of 64, < 32768)
LN_EPS = 1e-5
NCORES = 8
CH = 8                  # channels per core
BH = 12                 # chunks per stream batch
CALLB = 12              # chunks per dma_gather call (1536 descs; ring 8192)
REPLICA_GROUPS = [[0, 1, 2, 3, 4, 5, 6, 7]]


class Cfg:      # kept for test.py compatibility
    n_cores = NCORES


FULL = Cfg()


# ----------------------------------------------------------------------------
# Host-side preprocessing
# ----------------------------------------------------------------------------

LAST_META = None


def preprocess(cfg, x, edge_index, omega, proj, W1, W2, ln_gamma, ln_beta):
    src = np.asarray(edge_index[0], dtype=np.int64)
    dst = np.asarray(edge_index[1], dtype=np.int64)
    omega = np.asarray(omega, dtype=np.float32)
    x = np.asarray(x, dtype=np.float32)

    order = np.argsort(dst, kind="stable")
    src_s, dst_s, om_s = src[order], dst[order], omega[order]
    bounds = np.searchsorted(dst_s, [r * QROWS for r in range(NCORES + 1)])

    # per-core edge data in local coordinates
    cores = []
    for r in range(NCORES):
        b0, b1 = bounds[r], bounds[r + 1]
        s_r = src_s[b0:b1]
        d_r = dst_s[b0:b1] - r * QROWS
        jl = d_r // GRP
        cores.append({"src": s_r, "jl": jl, "col": d_r - jl * GRP,
                      "om": om_s[b0:b1],
                      "total": np.bincount(jl, minlength=QGRP)})

    # per-core group permutation: slot s hosts the s-th heaviest group
    perm = [np.argsort(-c["total"], kind="stable") for c in cores]
    inv_perm = []
    for r in range(NCORES):
        ip = np.empty(QGRP, np.int64)
        ip[perm[r]] = np.arange(QGRP)
        inv_perm.append(ip)
        cores[r]["slot"] = ip[cores[r]["jl"]]

    # h1 (slot/half-major) table row for each node-order padded index
    def h1_rows(n):
        r = n // QROWS
        q = n % QROWS
        jl = q // GRP
        w = q - jl * GRP
        qs = np.stack([inv_perm[rr] for rr in range(NCORES)])[r, jl] * GRP + w
        p = qs // HR
        return p * (NCORES * HR) + r * HR + (qs - p * HR)

    # per layer: table row of each edge's src
    layers = []     # layers[L][r][half] dicts
    for L in range(2):
        per_r = []
        for r in range(NCORES):
            c = cores[r]
            rows = c["src"] if L == 0 else h1_rows(c["src"])
            lo_mask = rows < LO_ROWS
            hs = []
            for m in (lo_mask, ~lo_mask):
                sel = np.nonzero(m)[0]
                # order within half by slot (stable)
                o = np.argsort(c["slot"][sel], kind="stable")
                sel = sel[o]
                slot = c["slot"][sel]
                hs.append({"rows": rows[sel], "slot": slot,
                           "col": c["col"][sel], "om": c["om"][sel],
                           "counts": np.bincount(slot, minlength=QGRP)})
            per_r.append(hs)
        layers.append(per_r)

    # SPMD-common chunk layout per layer/half (max over cores per slot)
    chunks = []  # chunks[L][half]
    for L in range(2):
        ch2 = []
        for h in range(2):
            cnt = np.stack([layers[L][r][h]["counts"] for r in range(NCORES)])
            ch2.append(((cnt.max(axis=0) + 127) // 128).astype(np.int64))
        tot0 = (ch2[0] + ch2[1]) == 0
        ch2[0] = np.maximum(ch2[0], tot0.astype(np.int64))
        chunks.append(ch2)

    # idx16 + S for each (layer, half, core)
    idx16 = [[[None] * NCORES for _ in range(2)] for _ in range(2)]
    S_arr = [[[None] * NCORES for _ in range(2)] for _ in range(2)]
    for L in range(2):
        for h in range(2):
            padded = chunks[L][h] * 128
            starts = np.concatenate([[0], np.cumsum(padded)[:-1]])
            Ln = int(padded.sum())
            for r in range(NCORES):
                hd = layers[L][r][h]
                cum = np.concatenate([[0], np.cumsum(hd["counts"])[:-1]])
                rank = (np.arange(len(hd["rows"]))
                        - np.repeat(cum, hd["counts"]))
                pos = starts[hd["slot"]] + rank
                idx_vals = np.zeros(Ln, np.int64)
                idx_vals[pos] = hd["rows"] - (0 if h == 0 else LO_ROWS)
                assert idx_vals.min() >= 0 and idx_vals.max() < 32768
                idx16[L][h][r] = np.ascontiguousarray(
                    idx_vals.astype(np.int16).reshape(-1, 16).T)
                S = np.zeros((Ln, CH, GRP), NPBF16)
                omv = hd["om"].astype(NPBF16)
                for c in range(CH):
                    S[pos, c, hd["col"]] = omv[:, c]
                S_arr[L][h][r] = np.ascontiguousarray(
                    S.reshape(Ln // 128, 128, CH, GRP).transpose(1, 0, 2, 3))

    xT = np.zeros((D, NPAD), NPBF16)
    xT[:, :N] = x.T.astype(NPBF16)

    skip_affine = bool(np.all(np.asarray(ln_gamma) == 1.0)
                       and np.all(np.asarray(ln_beta) == 0.0))

    proj_f = np.ascontiguousarray(
        np.asarray(proj, np.float32).reshape(D, C * H).astype(NPBF16))
    W1_f = np.ascontiguousarray(
        np.asarray(W1, np.float32).transpose(1, 0, 2).reshape(H, C * H)
        .astype(NPBF16))
    W2_f = np.ascontiguousarray(
        np.asarray(W2, np.float32).transpose(1, 0, 2).reshape(H, C * H)
        .astype(NPBF16))

    per_core = []
    for r in range(NCORES):
        m = {"xT": xT, "proj": proj_f, "W1": W1_f, "W2": W2_f,
             "idx_lo_a": idx16[0][0][r], "idx_hi_a": idx16[0][1][r],
             "idx_lo_b": idx16[1][0][r], "idx_hi_b": idx16[1][1][r],
             "S_lo_a": S_arr[0][0][r], "S_hi_a": S_arr[0][1][r],
             "S_lo_b": S_arr[1][0][r], "S_hi_b": S_arr[1][1][r]}
        if not skip_affine:
            m["gamma"] = np.asarray(ln_gamma, np.float32).reshape(1, H)
            m["beta"] = np.asarray(ln_beta, np.float32).reshape(1, H)
        per_core.append(m)

    meta = {
        "lo_a": [int(v) for v in chunks[0][0]],
        "hi_a": [int(v) for v in chunks[0][1]],
        "lo_b": [int(v) for v in chunks[1][0]],
        "hi_b": [int(v) for v in chunks[1][1]],
        "inv_perm": inv_perm,
    }
    global LAST_META
    LAST_META = meta
    return per_core, meta, None, skip_affine


# ----------------------------------------------------------------------------
# Bass program
# ----------------------------------------------------------------------------

class GatherStream:
    """Streams gather tiles + S tiles for one (layer, half)."""

    def __init__(self, nc, name, idx_dram, S_dram, n_chunks, table_view,
                 gpool, spool, ipool):
        self.nc, self.name = nc, name
        self.idx_dram, self.S_dram = idx_dram, S_dram
        self.n_chunks = n_chunks
        self.table_view = table_view
        self.gpool, self.spool, self.ipool = gpool, spool, ipool
        self.cur_bt = -1
        self.gtile = None
        self.stile = None

    def _issue(self, bt):
        nc = self.nc
        b0 = bt * BH
        B = min(BH, self.n_chunks - b0)
        itile = self.ipool.tile([128, BH * 8], mybir.dt.int16, tag="i")
        idx_src = self.idx_dram[:, b0 * 8: b0 * 8 + B * 8]
        bcast = bass.AP(tensor=idx_src.tensor, offset=idx_src.offset,
                        ap=[[0, 8]] + idx_src.ap)
        nc.sync.dma_start(out=itile[:, :B * 8], in_=bcast)

        self.gtile = self.gpool.tile([128, BH, CH * H], BF16, tag="g")
        for cb in range(0, B, CALLB):
            nb = min(CALLB, B - cb)
            nc.gpsimd.dma_gather(
                out_ap=self.gtile[:, cb:cb + nb, :], in_ap=self.table_view,
                idxs_ap=itile[:, cb * 8:(cb + nb) * 8],
                num_idxs=nb * 128, num_idxs_reg=nb * 128,
                elem_size=CH * H, single_packet=False)

        self.stile = self.spool.tile([128, BH, CH, GRP], BF16, tag="s")
        nc.sync.dma_start(out=self.stile[:, :B, :, :],
                          in_=self.S_dram[:, b0:b0 + B, :, :])
        self.cur_bt = bt

    def chunk(self, ci):
        bt, off = divmod(ci, BH)
        if bt != self.cur_bt:
            assert bt == self.cur_bt + 1
            self._issue(bt)
        return self.gtile[:, off, :], self.stile[:, off, :, :]


def build_program(cfg, meta, _unused, skip_affine, num_devices=8):
    nc = bacc.Bacc("TRN2", target_bir_lowering=False, debug=False,
                   num_devices=num_devices,
                   dynamic_dma_scratch_size=32768)
    NLa, NHa = sum(meta["lo_a"]), sum(meta["hi_a"])
    NLb, NHb = sum(meta["lo_b"]), sum(meta["hi_b"])

    xT = nc.dram_tensor("xT", [D, NPAD], BF16, kind="ExternalInput").ap()
    proj = nc.dram_tensor("proj", [D, CH * H], BF16, kind="ExternalInput").ap()
    W1 = nc.dram_tensor("W1", [H, CH * H], BF16, kind="ExternalInput").ap()
    W2 = nc.dram_tensor("W2", [H, CH * H], BF16, kind="ExternalInput").ap()

    def decl(name, nchunks, dt2, shape_fn):
        return nc.dram_tensor(name, shape_fn(nchunks), dt2,
                              kind="ExternalInput").ap()

    idx_sh = lambda n: [16, n * 8]
    s_sh = lambda n: [128, n, CH, GRP]
    idx_lo_a = decl("idx_lo_a", NLa, mybir.dt.int16, idx_sh)
    idx_hi_a = decl("idx_hi_a", NHa, mybir.dt.int16, idx_sh)
    idx_lo_b = decl("idx_lo_b", NLb, mybir.dt.int16, idx_sh)
    idx_hi_b = decl("idx_hi_b", NHb, mybir.dt.int16, idx_sh)
    S_lo_a = decl("S_lo_a", NLa, BF16, s_sh)
    S_hi_a = decl("S_hi_a", NHa, BF16, s_sh)
    S_lo_b = decl("S_lo_b", NLb, BF16, s_sh)
    S_hi_b = decl("S_hi_b", NHb, BF16, s_sh)

    out = nc.dram_tensor("out", [QROWS, CH * H], F32,
                         kind="ExternalOutput").ap()
    if not skip_affine:
        gamma = nc.dram_tensor("gamma", [1, H], F32, kind="ExternalInput").ap()
        beta = nc.dram_tensor("beta", [1, H], F32, kind="ExternalInput").ap()

    with tile.TileContext(nc) as tc:
        with (
            tc.tile_pool(name="dram", bufs=1, space="DRAM") as dpool,
            tc.tile_pool(name="singles", bufs=1) as singles,
            tc.tile_pool(name="xt", bufs=3) as xtpool,
            tc.tile_pool(name="pproj", bufs=2, space="PSUM") as pproj,
            tc.tile_pool(name="projsb", bufs=3) as projsb,
            tc.tile_pool(name="glo", bufs=2) as glo,
            tc.tile_pool(name="ghi", bufs=2) as ghi,
            tc.tile_pool(name="slo", bufs=2) as slo,
            tc.tile_pool(name="shi", bufs=2) as shi,
            tc.tile_pool(name="ilo", bufs=2) as ilo,
            tc.tile_pool(name="ihi", bufs=2) as ihi,
            tc.tile_pool(name="paggT", bufs=2, space="PSUM") as paggT,
            tc.tile_pool(name="pout", bufs=2, space="PSUM") as pout,
            tc.tile_pool(name="convsb", bufs=3) as convsb,
            tc.tile_pool(name="ob", bufs=3) as obpool,
            tc.tile_pool(name="ln", bufs=6) as lnpool,
        ):
            h0 = dpool.tile([NPAD, CH * H], BF16)
            h1_mine = dpool.tile([QROWS, CH * H], BF16)
            h1_full = dpool.tile([NPAD, CH * H], BF16)

            eps_t = singles.tile([128, 1], F32)
            nc.vector.memset(eps_t, LN_EPS)
            proj_t = singles.tile([128, D // 128, CH * H], BF16)
            nc.sync.dma_start(out=proj_t[:],
                              in_=proj.rearrange("(k p) h -> p k h", p=128))
            W1_t = singles.tile([H, CH * H], BF16)
            nc.sync.dma_start(out=W1_t[:], in_=W1[:])
            W2_t = singles.tile([H, CH * H], BF16)
            nc.sync.dma_start(out=W2_t[:], in_=W2[:])
            if not skip_affine:
                gamma_t = singles.tile([GRP, H], F32)
                nc.sync.dma_start(out=gamma_t[:], in_=bass.AP(
                    tensor=gamma.tensor, offset=gamma.offset,
                    ap=[[0, GRP]] + gamma.ap[1:]))
                beta_t = singles.tile([GRP, H], F32)
                nc.sync.dma_start(out=beta_t[:], in_=bass.AP(
                    tensor=beta.tensor, offset=beta.offset,
                    ap=[[0, GRP]] + beta.ap[1:]))

            # ---------------- phase A: h0 = x @ proj (full table) ----------
            # 2048-col batches; four 128-row result tiles packed into one
            # [128, 4, 512] SBUF tile and written with a single DMA per
            # 512 table rows (cuts the per-write HWDGE setup+sem chain 4x).
            KCH = D // 128
            COLB = 2048
            WPACK = 4
            ti = 0
            for c0 in range(0, NPAD, COLB):
                cb = min(COLB, NPAD - c0)
                xts = []
                for k in range(KCH):
                    xt_t = xtpool.tile([128, COLB], BF16, tag=f"xt{k}")
                    nc.sync.dma_start(out=xt_t[:, :cb],
                                      in_=xT[k * 128:(k + 1) * 128,
                                             c0:c0 + cb])
                    xts.append(xt_t)
                for w0 in range(0, cb, WPACK * 128):
                    wrows = min(WPACK * 128, cb - w0)
                    nw = wrows // 128
                    sb = projsb.tile([128, WPACK, CH * H], BF16, tag="sb")
                    for j in range(nw):
                        t0 = w0 + j * 128
                        ps = pproj.tile([128, CH * H], F32)
                        for k in range(KCH):
                            nc.tensor.matmul(out=ps[:],
                                             lhsT=xts[k][:, t0:t0 + 128],
                                             rhs=proj_t[:, k, :],
                                             start=(k == 0),
                                             stop=(k == KCH - 1))
                        if ti % 2 == 0:
                            nc.scalar.activation(
                                out=sb[:, j, :], in_=ps[:],
                                func=mybir.ActivationFunctionType.Copy)
                        else:
                            nc.vector.tensor_copy(out=sb[:, j, :], in_=ps[:])
                        ti += 1
                    dst_rows = h0[c0 + w0:c0 + w0 + wrows, :]
                    nc.sync.dma_start(
                        out=dst_rows.rearrange("(j p) f -> p j f", p=128),
                        in_=sb[:, :nw, :])
                    ti += 1

            # ---------------- conv layers ----------------
            def conv_layer(lname, table, idxlo, idxhi, Slo, Shi, NLx, NHx,
                           lo_chunks, hi_chunks, W_t, dst, dst_dt, relu):
                tlo = table[0:LO_ROWS, :]
                thi = table[LO_ROWS:NPAD, :]
                s_lo = GatherStream(nc, f"lo{lname}", idxlo, Slo, NLx, tlo,
                                    glo, slo, ilo)
                s_hi = GatherStream(nc, f"hi{lname}", idxhi, Shi, NHx, thi,
                                    ghi, shi, ihi)
                ci_lo = ci_hi = 0
                for g in range(QGRP):
                    nlo, nhi = lo_chunks[g], hi_chunks[g]
                    total = nlo + nhi
                    assert total > 0
                    agg = paggT.tile([H, CH * GRP], F32, tag="agg")
                    done = 0
                    for st, nch, ci0 in ((s_lo, nlo, ci_lo),
                                         (s_hi, nhi, ci_hi)):
                        for j in range(nch):
                            g_ap, s_ap = st.chunk(ci0 + j)
                            for c in range(CH):
                                nc.tensor.matmul(
                                    out=agg[:, c * GRP:(c + 1) * GRP],
                                    lhsT=g_ap[:, c * H:(c + 1) * H],
                                    rhs=s_ap[:, c, :],
                                    start=(done == 0 and c == 0),
                                    stop=(done == total - 1 and c == CH - 1))
                            done += 1
                    ci_lo += nlo
                    ci_hi += nhi

                    aggsb = convsb.tile([H, CH * GRP], BF16, tag="aggsb")
                    nc.scalar.activation(
                        out=aggsb[:], in_=agg[:],
                        func=mybir.ActivationFunctionType.Copy)
                    po = pout.tile([GRP, CH * H], F32, tag="po")
                    for c in range(CH):
                        nc.tensor.matmul(out=po[:, c * H:(c + 1) * H],
                                         lhsT=aggsb[:, c * GRP:(c + 1) * GRP],
                                         rhs=W_t[:, c * H:(c + 1) * H],
                                         start=(c == 0), stop=(c == CH - 1))
                    obrow = obpool.tile([GRP, CH * H], dst_dt, tag="obrow")
                    for c in range(CH):
                        poc = po[:, c * H:(c + 1) * H]
                        stats = lnpool.tile([GRP, 6], F32, tag=f"st{c}")
                        nc.vector.bn_stats(out=stats[:], in_=poc)
                        mv = lnpool.tile([GRP, 2], F32, tag=f"mv{c}")
                        nc.vector.bn_aggr(out=mv[:], in_=stats[:])
                        rstd = lnpool.tile([GRP, 1], F32, tag=f"rs{c}")
                        nc.scalar.activation(
                            out=rstd[:], in_=mv[:, 1:2],
                            func=mybir.ActivationFunctionType.Sqrt,
                            bias=eps_t[:GRP, :], scale=1.0)
                        nc.vector.reciprocal(out=rstd[:], in_=rstd[:])
                        nmr = lnpool.tile([GRP, 1], F32, tag=f"nm{c}")
                        nc.vector.tensor_scalar(
                            out=nmr[:], in0=mv[:, 0:1],
                            scalar1=rstd[:], scalar2=-1.0,
                            op0=mybir.AluOpType.mult,
                            op1=mybir.AluOpType.mult)
                        if skip_affine:
                            nc.scalar.activation(
                                out=obrow[:, c * H:(c + 1) * H], in_=poc,
                                func=(mybir.ActivationFunctionType.Relu if relu
                                      else mybir.ActivationFunctionType.Identity),
                                bias=nmr[:], scale=rstd[:])
                        else:
                            ob = convsb.tile([GRP, H], F32, tag=f"eb{c}")
                            nc.scalar.activation(
                                out=ob[:], in_=poc,
                                func=mybir.ActivationFunctionType.Identity,
                                bias=nmr[:], scale=rstd[:])
                            nc.vector.tensor_mul(out=ob[:], in0=ob[:],
                                                 in1=gamma_t[:])
                            nc.vector.tensor_add(out=ob[:], in0=ob[:],
                                                 in1=beta_t[:])
                            if relu:
                                nc.vector.tensor_scalar_max(
                                    out=obrow[:, c * H:(c + 1) * H],
                                    in0=ob[:], scalar1=0.0)
                            else:
                                nc.vector.tensor_copy(
                                    out=obrow[:, c * H:(c + 1) * H], in_=ob[:])
                    nc.sync.dma_start(out=dst[g * GRP:(g + 1) * GRP, :],
                                      in_=obrow[:])

            conv_layer("a", h0, idx_lo_a, idx_hi_a, S_lo_a, S_hi_a, NLa, NHa,
                       meta["lo_a"], meta["hi_a"], W1_t, h1_mine, BF16,
                       relu=True)
            # Two contiguous AllGathers: part p gathers rows [p*HR,(p+1)*HR)
            # of every eighth into the half-major region of h1_full.
            for p in range(2):
                nc.gpsimd.collective_compute(
                    "AllGather", mybir.AluOpType.bypass,
                    replica_groups=REPLICA_GROUPS,
                    ins=[h1_mine[p * HR:(p + 1) * HR, :].opt()],
                    outs=[h1_full[p * NCORES * HR:(p + 1) * NCORES * HR,
                                  :].opt()])
            conv_layer("b", h1_full, idx_lo_b, idx_hi_b, S_lo_b, S_hi_b,
                       NLb, NHb, meta["lo_b"], meta["hi_b"], W2_t, out, F32,
                       relu=False)

    nc.compile()
    return nc


# ----------------------------------------------------------------------------
# Entry point
# ----------------------------------------------------------------------------

def assemble(results, inv_perm=None):
    if inv_perm is None:
        inv_perm = LAST_META["inv_perm"]
    full = np.empty((N, C, H), np.float32)
    for k in range(NCORES):
        r0 = k * QROWS
        rows = min(QROWS, N - r0)
        if rows <= 0:
            continue
        o = results[k]["out"]                   # [QROWS, CH*H] f32 slot order
        q = np.arange(rows)
        jl = q // GRP
        w = q - jl * GRP
        slot_rows = inv_perm[k][jl] * GRP + w
        full[r0:r0 + rows, :, :] = o[slot_rows].reshape(rows, CH, H)
    return full


def kernel(x, edge_index, omega, proj, W1, W2, ln_gamma, ln_beta):
    per_core, meta, _, skip_affine = preprocess(
        FULL, x, edge_index, omega, proj, W1, W2, ln_gamma, ln_beta)
    nc = build_program(FULL, meta, None, skip_affine, num_devices=NCORES)
    res = bass_utils.run_bass_kernel_spmd(
        nc, per_core, core_ids=list(range(NCORES)))
    return np.ascontiguousarray(
        assemble(res.results, meta["inv_perm"]), dtype=np.float32)



# revision 14
# speedup vs baseline: 1.2053x; 1.2053x over previous
"""Trainium2 Bass kernel for nn_DisentangledGraphConvEncoder (octo v4).

Sharding: core k owns dst eighth k (6272 padded rows) and computes ALL 8
channels for those dsts. Tables are [NPAD, 512] bf16 (8 ch x 64 feats = 1KB
rows) so each gather descriptor moves 1KB.

v4 refinements over v3:
- Per-core groups are permuted by descending edge count before slot
  assignment, so the SPMD-common (max-over-cores) chunk layout wastes ~12%
  instead of ~24%.
- The h1 table uses a half-major layout (half p of every eighth, rank-major)
  so the inter-layer AllGather splits into two contiguous-output collectives;
  part 1 overlaps the tail of layer a.
- Layer a gathers from h0 in node order; layer b gathers from h1 in
  slot/half-major order — separate idx16 + S streams per layer.
"""

import numpy as np
import ml_dtypes

import concourse.bass as bass
import concourse.bacc as bacc
import concourse.tile as tile
from concourse import mybir
from concourse import bass_utils

F32 = mybir.dt.float32
BF16 = mybir.dt.bfloat16
NPBF16 = ml_dtypes.bfloat16

N = 50000
E = 800000
D = 256
C = 8
H = 64
GRP = 64
NPAD = 50176            # 784 groups of 64; divisible by 8*64
QROWS = NPAD // 8       # 6272 rows per dst eighth
QGRP = QROWS // GRP     # 98 groups per eighth
NPARTS = 4              # inter-layer AllGather parts
HR = QROWS // NPARTS    # 1568 rows: AG part size per rank
LO_ROWS = 25088         # = 8*2*HR: lo gather-table rows (mult of 64, < 32768)
LN_EPS = 1e-5
NCORES = 8
CH = 8                  # channels per core
BH = 12                 # chunks per stream batch
CALLB = 6               # chunks per dma_gather call (768 descs)
NSWQ = 4                # SWDGE queues; gather calls round-robin across them
REPLICA_GROUPS = [[0, 1, 2, 3, 4, 5, 6, 7]]


class Cfg:      # kept for test.py compatibility
    n_cores = NCORES


FULL = Cfg()


# ----------------------------------------------------------------------------
# Host-side preprocessing
# ----------------------------------------------------------------------------

LAST_META = None


def preprocess(cfg, x, edge_index, omega, proj, W1, W2, ln_gamma, ln_beta):
    src = np.asarray(edge_index[0], dtype=np.int64)
    dst = np.asarray(edge_index[1], dtype=np.int64)
    omega = np.asarray(omega, dtype=np.float32)
    x = np.asarray(x, dtype=np.float32)

    order = np.argsort(dst, kind="stable")
    src_s, dst_s, om_s = src[order], dst[order], omega[order]
    bounds = np.searchsorted(dst_s, [r * QROWS for r in range(NCORES + 1)])

    # per-core edge data in local coordinates
    cores = []
    for r in range(NCORES):
        b0, b1 = bounds[r], bounds[r + 1]
        s_r = src_s[b0:b1]
        d_r = dst_s[b0:b1] - r * QROWS
        jl = d_r // GRP
        cores.append({"src": s_r, "jl": jl, "col": d_r - jl * GRP,
                      "om": om_s[b0:b1],
                      "total": np.bincount(jl, minlength=QGRP)})

    # per-core group permutation: slot s hosts the s-th heaviest group
    perm = [np.argsort(-c["total"], kind="stable") for c in cores]
    inv_perm = []
    for r in range(NCORES):
        ip = np.empty(QGRP, np.int64)
        ip[perm[r]] = np.arange(QGRP)
        inv_perm.append(ip)
        cores[r]["slot"] = ip[cores[r]["jl"]]

    # h1 (slot/half-major) table row for each node-order padded index
    def h1_rows(n):
        r = n // QROWS
        q = n % QROWS
        jl = q // GRP
        w = q - jl * GRP
        qs = np.stack([inv_perm[rr] for rr in range(NCORES)])[r, jl] * GRP + w
        p = qs // HR
        return p * (NCORES * HR) + r * HR + (qs - p * HR)

    # per layer: table row of each edge's src
    layers = []     # layers[L][r][half] dicts
    for L in range(2):
        per_r = []
        for r in range(NCORES):
            c = cores[r]
            rows = c["src"] if L == 0 else h1_rows(c["src"])
            lo_mask = rows < LO_ROWS
            hs = []
            for m in (lo_mask, ~lo_mask):
                sel = np.nonzero(m)[0]
                # order within half by slot (stable)
                o = np.argsort(c["slot"][sel], kind="stable")
                sel = sel[o]
                slot = c["slot"][sel]
                hs.append({"rows": rows[sel], "slot": slot,
                           "col": c["col"][sel], "om": c["om"][sel],
                           "counts": np.bincount(slot, minlength=QGRP)})
            per_r.append(hs)
        layers.append(per_r)

    # SPMD-common chunk layout per layer/half (max over cores per slot)
    chunks = []  # chunks[L][half]
    for L in range(2):
        ch2 = []
        for h in range(2):
            cnt = np.stack([layers[L][r][h]["counts"] for r in range(NCORES)])
            ch2.append(((cnt.max(axis=0) + 127) // 128).astype(np.int64))
        tot0 = (ch2[0] + ch2[1]) == 0
        ch2[0] = np.maximum(ch2[0], tot0.astype(np.int64))
        chunks.append(ch2)

    # idx16 + S for each (layer, half, core)
    idx16 = [[[None] * NCORES for _ in range(2)] for _ in range(2)]
    S_arr = [[[None] * NCORES for _ in range(2)] for _ in range(2)]
    for L in range(2):
        for h in range(2):
            padded = chunks[L][h] * 128
            starts = np.concatenate([[0], np.cumsum(padded)[:-1]])
            Ln = int(padded.sum())
            for r in range(NCORES):
                hd = layers[L][r][h]
                cum = np.concatenate([[0], np.cumsum(hd["counts"])[:-1]])
                rank = (np.arange(len(hd["rows"]))
                        - np.repeat(cum, hd["counts"]))
                pos = starts[hd["slot"]] + rank
                idx_vals = np.zeros(Ln, np.int64)
                idx_vals[pos] = hd["rows"] - (0 if h == 0 else LO_ROWS)
                assert idx_vals.min() >= 0 and idx_vals.max() < 32768
                idx16[L][h][r] = np.ascontiguousarray(
                    idx_vals.astype(np.int16).reshape(-1, 16).T)
                S = np.zeros((Ln, CH, GRP), NPBF16)
                omv = hd["om"].astype(NPBF16)
                for c in range(CH):
                    S[pos, c, hd["col"]] = omv[:, c]
                S_arr[L][h][r] = np.ascontiguousarray(
                    S.reshape(Ln // 128, 128, CH, GRP).transpose(1, 0, 2, 3))

    xT = np.zeros((D, NPAD), NPBF16)
    xT[:, :N] = x.T.astype(NPBF16)

    skip_affine = bool(np.all(np.asarray(ln_gamma) == 1.0)
                       and np.all(np.asarray(ln_beta) == 0.0))

    proj_f = np.ascontiguousarray(
        np.asarray(proj, np.float32).reshape(D, C * H).astype(NPBF16))
    W1_f = np.ascontiguousarray(
        np.asarray(W1, np.float32).transpose(1, 0, 2).reshape(H, C * H)
        .astype(NPBF16))
    W2_f = np.ascontiguousarray(
        np.asarray(W2, np.float32).transpose(1, 0, 2).reshape(H, C * H)
        .astype(NPBF16))

    per_core = []
    for r in range(NCORES):
        m = {"xT": xT, "proj": proj_f, "W1": W1_f, "W2": W2_f,
             "idx_lo_a": idx16[0][0][r], "idx_hi_a": idx16[0][1][r],
             "idx_lo_b": idx16[1][0][r], "idx_hi_b": idx16[1][1][r],
             "S_lo_a": S_arr[0][0][r], "S_hi_a": S_arr[0][1][r],
             "S_lo_b": S_arr[1][0][r], "S_hi_b": S_arr[1][1][r]}
        if not skip_affine:
            m["gamma"] = np.asarray(ln_gamma, np.float32).reshape(1, H)
            m["beta"] = np.asarray(ln_beta, np.float32).reshape(1, H)
        per_core.append(m)

    meta = {
        "lo_a": [int(v) for v in chunks[0][0]],
        "hi_a": [int(v) for v in chunks[0][1]],
        "lo_b": [int(v) for v in chunks[1][0]],
        "hi_b": [int(v) for v in chunks[1][1]],
        "inv_perm": inv_perm,
    }
    global LAST_META
    LAST_META = meta
    return per_core, meta, None, skip_affine


# ----------------------------------------------------------------------------
# Bass program
# ----------------------------------------------------------------------------

QCTR = [0]   # global gather-call counter for SWDGE queue round-robin


class GatherStream:
    """Streams gather tiles + S tiles for one (layer, half)."""

    def __init__(self, nc, name, idx_dram, S_dram, n_chunks, table_view,
                 gpool, spool, ipool):
        self.nc, self.name = nc, name
        self.idx_dram, self.S_dram = idx_dram, S_dram
        self.n_chunks = n_chunks
        self.table_view = table_view
        self.gpool, self.spool, self.ipool = gpool, spool, ipool
        self.cur_bt = -1
        self.gtile = None
        self.stile = None

    def _issue(self, bt):
        nc = self.nc
        b0 = bt * BH
        B = min(BH, self.n_chunks - b0)
        itile = self.ipool.tile([128, BH * 8], mybir.dt.int16, tag="i")
        idx_src = self.idx_dram[:, b0 * 8: b0 * 8 + B * 8]
        bcast = bass.AP(tensor=idx_src.tensor, offset=idx_src.offset,
                        ap=[[0, 8]] + idx_src.ap)
        nc.sync.dma_start(out=itile[:, :B * 8], in_=bcast)

        self.gtile = self.gpool.tile([128, BH, CH * H], BF16, tag="g")
        for cb in range(0, B, CALLB):
            nb = min(CALLB, B - cb)
            nc.gpsimd.dma_gather(
                out_ap=self.gtile[:, cb:cb + nb, :], in_ap=self.table_view,
                idxs_ap=itile[:, cb * 8:(cb + nb) * 8],
                num_idxs=nb * 128, num_idxs_reg=nb * 128,
                elem_size=CH * H, single_packet=False,
                queue_num=QCTR[0] % NSWQ)
            QCTR[0] += 1

        self.stile = self.spool.tile([128, BH, CH, GRP], BF16, tag="s")
        nc.sync.dma_start(out=self.stile[:, :B, :, :],
                          in_=self.S_dram[:, b0:b0 + B, :, :])
        self.cur_bt = bt

    def chunk(self, ci):
        bt, off = divmod(ci, BH)
        if bt != self.cur_bt:
            assert bt == self.cur_bt + 1
            self._issue(bt)
        return self.gtile[:, off, :], self.stile[:, off, :, :]


def build_program(cfg, meta, _unused, skip_affine, num_devices=8):
    QCTR[0] = 0
    nc = bacc.Bacc("TRN2", target_bir_lowering=False, debug=False,
                   num_devices=num_devices,
                   num_swdge_queues=NSWQ,
                   dynamic_dma_scratch_size=32768)
    NLa, NHa = sum(meta["lo_a"]), sum(meta["hi_a"])
    NLb, NHb = sum(meta["lo_b"]), sum(meta["hi_b"])

    xT = nc.dram_tensor("xT", [D, NPAD], BF16, kind="ExternalInput").ap()
    proj = nc.dram_tensor("proj", [D, CH * H], BF16, kind="ExternalInput").ap()
    W1 = nc.dram_tensor("W1", [H, CH * H], BF16, kind="ExternalInput").ap()
    W2 = nc.dram_tensor("W2", [H, CH * H], BF16, kind="ExternalInput").ap()

    def decl(name, nchunks, dt2, shape_fn):
        return nc.dram_tensor(name, shape_fn(nchunks), dt2,
                              kind="ExternalInput").ap()

    idx_sh = lambda n: [16, n * 8]
    s_sh = lambda n: [128, n, CH, GRP]
    idx_lo_a = decl("idx_lo_a", NLa, mybir.dt.int16, idx_sh)
    idx_hi_a = decl("idx_hi_a", NHa, mybir.dt.int16, idx_sh)
    idx_lo_b = decl("idx_lo_b", NLb, mybir.dt.int16, idx_sh)
    idx_hi_b = decl("idx_hi_b", NHb, mybir.dt.int16, idx_sh)
    S_lo_a = decl("S_lo_a", NLa, BF16, s_sh)
    S_hi_a = decl("S_hi_a", NHa, BF16, s_sh)
    S_lo_b = decl("S_lo_b", NLb, BF16, s_sh)
    S_hi_b = decl("S_hi_b", NHb, BF16, s_sh)

    out = nc.dram_tensor("out", [QROWS, CH * H], F32,
                         kind="ExternalOutput").ap()
    if not skip_affine:
        gamma = nc.dram_tensor("gamma", [1, H], F32, kind="ExternalInput").ap()
        beta = nc.dram_tensor("beta", [1, H], F32, kind="ExternalInput").ap()

    import contextlib
    with tile.TileContext(nc) as tc:
        with contextlib.ExitStack() as _ctx:
            def _pool(**kw):
                return _ctx.enter_context(tc.tile_pool(**kw))
            dpool = _pool(name="dram", bufs=1, space="DRAM")
            singles = _pool(name="singles", bufs=1)
            xtpool = _pool(name="xt", bufs=3)
            pproj = _pool(name="pproj", bufs=2, space="PSUM")
            projsb = _pool(name="projsb", bufs=3)
            glo = _pool(name="glo", bufs=2)
            ghi = _pool(name="ghi", bufs=2)
            slo = _pool(name="slo", bufs=2)
            shi = _pool(name="shi", bufs=2)
            ilo = _pool(name="ilo", bufs=2)
            ihi = _pool(name="ihi", bufs=2)
            paggT = _pool(name="paggT", bufs=2, space="PSUM")
            pout = _pool(name="pout", bufs=2, space="PSUM")
            convsb = _pool(name="convsb", bufs=3)
            obpool = _pool(name="ob", bufs=3)
            lnpool = _pool(name="ln", bufs=6)
            sqpool = _pool(name="sq", bufs=2)
            h0_lo = dpool.tile([LO_ROWS, CH * H], BF16)
            h0_hi = dpool.tile([NPAD - LO_ROWS, CH * H], BF16)
            h1_mine = dpool.tile([QROWS, CH * H], BF16)
            h1_full = dpool.tile([NPAD, CH * H], BF16)

            eps_t = singles.tile([128, 1], F32)
            nc.vector.memset(eps_t, LN_EPS)
            proj_t = singles.tile([128, D // 128, CH * H], BF16)
            nc.sync.dma_start(out=proj_t[:],
                              in_=proj.rearrange("(k p) h -> p k h", p=128))
            W1_t = singles.tile([H, CH * H], BF16)
            nc.sync.dma_start(out=W1_t[:], in_=W1[:])
            W2_t = singles.tile([H, CH * H], BF16)
            nc.sync.dma_start(out=W2_t[:], in_=W2[:])
            if not skip_affine:
                gamma_t = singles.tile([GRP, H], F32)
                nc.sync.dma_start(out=gamma_t[:], in_=bass.AP(
                    tensor=gamma.tensor, offset=gamma.offset,
                    ap=[[0, GRP]] + gamma.ap[1:]))
                beta_t = singles.tile([GRP, H], F32)
                nc.sync.dma_start(out=beta_t[:], in_=bass.AP(
                    tensor=beta.tensor, offset=beta.offset,
                    ap=[[0, GRP]] + beta.ap[1:]))

            # ---------------- phase A: h0 = x @ proj (full table) ----------
            # 2048-col batches; four 128-row result tiles packed into one
            # [128, 4, 512] SBUF tile and written with a single DMA per
            # 512 table rows (cuts the per-write HWDGE setup+sem chain 4x).
            KCH = D // 128
            COLB = 2048
            WPACK = 4
            ti = 0
            for c0 in range(0, NPAD, COLB):
                cb = min(COLB, NPAD - c0)
                xts = []
                for k in range(KCH):
                    xt_t = xtpool.tile([128, COLB], BF16, tag=f"xt{k}")
                    nc.sync.dma_start(out=xt_t[:, :cb],
                                      in_=xT[k * 128:(k + 1) * 128,
                                             c0:c0 + cb])
                    xts.append(xt_t)
                for w0 in range(0, cb, WPACK * 128):
                    wrows = min(WPACK * 128, cb - w0)
                    nw = wrows // 128
                    sb = projsb.tile([128, WPACK, CH * H], BF16, tag="sb")
                    for j in range(nw):
                        t0 = w0 + j * 128
                        ps = pproj.tile([128, CH * H], F32)
                        for k in range(KCH):
                            nc.tensor.matmul(out=ps[:],
                                             lhsT=xts[k][:, t0:t0 + 128],
                                             rhs=proj_t[:, k, :],
                                             start=(k == 0),
                                             stop=(k == KCH - 1))
                        if ti % 2 == 0:
                            nc.scalar.activation(
                                out=sb[:, j, :], in_=ps[:],
                                func=mybir.ActivationFunctionType.Copy)
                        else:
                            nc.vector.tensor_copy(out=sb[:, j, :], in_=ps[:])
                        ti += 1
                    r0 = c0 + w0
                    if r0 < LO_ROWS:
                        dst_rows = h0_lo[r0:r0 + wrows, :]
                    else:
                        dst_rows = h0_hi[r0 - LO_ROWS:r0 - LO_ROWS + wrows, :]
                    nc.sync.dma_start(
                        out=dst_rows.rearrange("(j p) f -> p j f", p=128),
                        in_=sb[:, :nw, :])
                    ti += 1

            # ---------------- conv layers ----------------
            def conv_layer(lname, tlo, thi, idxlo, idxhi, Slo, Shi, NLx, NHx,
                           lo_chunks, hi_chunks, W_t, dst, dst_dt, relu):
                s_lo = GatherStream(nc, f"lo{lname}", idxlo, Slo, NLx, tlo,
                                    glo, slo, ilo)
                s_hi = GatherStream(nc, f"hi{lname}", idxhi, Shi, NHx, thi,
                                    ghi, shi, ihi)
                ci_lo = ci_hi = 0
                for g in range(QGRP):
                    nlo, nhi = lo_chunks[g], hi_chunks[g]
                    total = nlo + nhi
                    assert total > 0
                    agg = paggT.tile([H, CH * GRP], F32, tag="agg")
                    done = 0
                    for st, nch, ci0 in ((s_lo, nlo, ci_lo),
                                         (s_hi, nhi, ci_hi)):
                        for j in range(nch):
                            g_ap, s_ap = st.chunk(ci0 + j)
                            for c in range(CH):
                                nc.tensor.matmul(
                                    out=agg[:, c * GRP:(c + 1) * GRP],
                                    lhsT=g_ap[:, c * H:(c + 1) * H],
                                    rhs=s_ap[:, c, :],
                                    start=(done == 0 and c == 0),
                                    stop=(done == total - 1 and c == CH - 1))
                            done += 1
                    ci_lo += nlo
                    ci_hi += nhi

                    aggsb = convsb.tile([H, CH * GRP], BF16, tag="aggsb")
                    if g % 2 == 0:
                        nc.scalar.activation(
                            out=aggsb[:], in_=agg[:],
                            func=mybir.ActivationFunctionType.Copy)
                    else:
                        nc.vector.tensor_copy(out=aggsb[:], in_=agg[:])
                    po = pout.tile([GRP, CH * H], F32, tag="po")
                    for c in range(CH):
                        nc.tensor.matmul(out=po[:, c * H:(c + 1) * H],
                                         lhsT=aggsb[:, c * GRP:(c + 1) * GRP],
                                         rhs=W_t[:, c * H:(c + 1) * H],
                                         start=(c == 0), stop=(c == CH - 1))
                    obrow = obpool.tile([GRP, CH * H], dst_dt, tag="obrow")
                    # Copy po to SBUF (DVE cannot read PSUM on both ports),
                    # then batched per-channel LN stats via strided reduces.
                    posb = sqpool.tile([GRP, CH * H], F32, tag="posb")
                    if g % 2 == 1:
                        nc.scalar.activation(
                            out=posb[:], in_=po[:],
                            func=mybir.ActivationFunctionType.Copy)
                    else:
                        nc.vector.tensor_copy(out=posb[:], in_=po[:])
                    po3 = posb[:].rearrange("p (c h) -> p c h", c=CH)
                    sum_t = lnpool.tile([GRP, CH], F32, tag="s")
                    sq = sqpool.tile([GRP, CH, H], F32, tag="sq")
                    sumsq = lnpool.tile([GRP, CH], F32, tag="ss")
                    s2 = lnpool.tile([GRP, CH], F32, tag="s2")
                    q_t = lnpool.tile([GRP, CH], F32, tag="q")
                    rstd = lnpool.tile([GRP, CH], F32, tag="rs")
                    nmr = lnpool.tile([GRP, CH], F32, tag="nm")
                    nc.vector.tensor_reduce(out=sum_t[:], in_=po3,
                                            axis=mybir.AxisListType.X,
                                            op=mybir.AluOpType.add)
                    nc.vector.tensor_mul(out=sq[:], in0=po3, in1=po3)
                    nc.vector.tensor_reduce(out=sumsq[:], in_=sq[:],
                                            axis=mybir.AxisListType.X,
                                            op=mybir.AluOpType.add)
                    nc.vector.tensor_mul(out=s2[:], in0=sum_t[:],
                                         in1=sum_t[:])
                    # q = sumsq - sum^2/H = H*var; rstd = 1/sqrt(q/H + eps)
                    nc.vector.scalar_tensor_tensor(
                        out=q_t[:], in0=s2[:], scalar=float(-1.0 / H),
                        in1=sumsq[:], op0=mybir.AluOpType.mult,
                        op1=mybir.AluOpType.add)
                    nc.scalar.activation(
                        out=rstd[:], in_=q_t[:],
                        func=mybir.ActivationFunctionType.Sqrt,
                        bias=eps_t[:GRP, :], scale=float(1.0 / H))
                    nc.vector.reciprocal(out=rstd[:], in_=rstd[:])
                    # nmr = -mean * rstd = (sum * -1/H) * rstd
                    nc.vector.scalar_tensor_tensor(
                        out=nmr[:], in0=sum_t[:], scalar=float(-1.0 / H),
                        in1=rstd[:], op0=mybir.AluOpType.mult,
                        op1=mybir.AluOpType.mult)
                    for c in range(CH):
                        poc = posb[:, c * H:(c + 1) * H]
                        if skip_affine:
                            nc.scalar.activation(
                                out=obrow[:, c * H:(c + 1) * H], in_=poc,
                                func=(mybir.ActivationFunctionType.Relu if relu
                                      else mybir.ActivationFunctionType.Identity),
                                bias=nmr[:, c:c + 1], scale=rstd[:, c:c + 1])
                        else:
                            ob = convsb.tile([GRP, H], F32, tag=f"eb{c}")
                            nc.scalar.activation(
                                out=ob[:], in_=poc,
                                func=mybir.ActivationFunctionType.Identity,
                                bias=nmr[:, c:c + 1], scale=rstd[:, c:c + 1])
                            nc.vector.tensor_mul(out=ob[:], in0=ob[:],
                                                 in1=gamma_t[:])
                            nc.vector.tensor_add(out=ob[:], in0=ob[:],
                                                 in1=beta_t[:])
                            if relu:
                                nc.vector.tensor_scalar_max(
                                    out=obrow[:, c * H:(c + 1) * H],
                                    in0=ob[:], scalar1=0.0)
                            else:
                                nc.vector.tensor_copy(
                                    out=obrow[:, c * H:(c + 1) * H], in_=ob[:])
                    nc.sync.dma_start(out=dst[g * GRP:(g + 1) * GRP, :],
                                      in_=obrow[:])

            conv_layer("a", h0_lo[:, :], h0_hi[:, :],
                       idx_lo_a, idx_hi_a, S_lo_a, S_hi_a, NLa, NHa,
                       meta["lo_a"], meta["hi_a"], W1_t, h1_mine, BF16,
                       relu=True)
            # NPARTS contiguous AllGathers: part p gathers rows
            # [p*HR,(p+1)*HR) of every eighth into the part-major region of
            # h1_full; early parts overlap the tail of layer a.
            for p in range(NPARTS):
                nc.gpsimd.collective_compute(
                    "AllGather", mybir.AluOpType.bypass,
                    replica_groups=REPLICA_GROUPS,
                    ins=[h1_mine[p * HR:(p + 1) * HR, :].opt()],
                    outs=[h1_full[p * NCORES * HR:(p + 1) * NCORES * HR,
                                  :].opt()])
            conv_layer("b", h1_full[0:LO_ROWS, :], h1_full[LO_ROWS:NPAD, :],
                       idx_lo_b, idx_hi_b, S_lo_b, S_hi_b,
                       NLb, NHb, meta["lo_b"], meta["hi_b"], W2_t, out, F32,
                       relu=False)

    nc.compile()
    return nc


# ----------------------------------------------------------------------------
# Entry point
# ----------------------------------------------------------------------------

def assemble(results, inv_perm=None):
    if inv_perm is None:
        inv_perm = LAST_META["inv_perm"]
    full = np.empty((N, C, H), np.float32)
    for k in range(NCORES):
        r0 = k * QROWS
        rows = min(QROWS, N - r0)
        if rows <= 0:
            continue
        o = results[k]["out"]                   # [QROWS, CH*H] f32 slot order
        q = np.arange(rows)
        jl = q // GRP
        w = q - jl * GRP
        slot_rows = inv_perm[k][jl] * GRP + w
        full[r0:r0 + rows, :, :] = o[slot_rows].reshape(rows, CH, H)
    return full


def kernel(x, edge_index, omega, proj, W1, W2, ln_gamma, ln_beta):
    per_core, meta, _, skip_affine = preprocess(
        FULL, x, edge_index, omega, proj, W1, W2, ln_gamma, ln_beta)
    nc = build_program(FULL, meta, None, skip_affine, num_devices=NCORES)
    res = bass_utils.run_bass_kernel_spmd(
        nc, per_core, core_ids=list(range(NCORES)))
    return np.ascontiguousarray(
        assemble(res.results, meta["inv_perm"]), dtype=np.float32)



# revision 29
# speedup vs baseline: 1.3977x; 1.1596x over previous
"""Trainium2 Bass kernel for nn_DisentangledGraphConvEncoder (octo v4).

Sharding: core k owns dst eighth k (6272 padded rows) and computes ALL 8
channels for those dsts. Tables are [NPAD, 512] bf16 (8 ch x 64 feats = 1KB
rows) so each gather descriptor moves 1KB.

v4 refinements over v3:
- Per-core groups are permuted by descending edge count before slot
  assignment, so the SPMD-common (max-over-cores) chunk layout wastes ~12%
  instead of ~24%.
- The h1 table uses a half-major layout (half p of every eighth, rank-major)
  so the inter-layer AllGather splits into two contiguous-output collectives;
  part 1 overlaps the tail of layer a.
- Layer a gathers from h0 in node order; layer b gathers from h1 in
  slot/half-major order — separate idx16 + S streams per layer.
"""

import numpy as np
import ml_dtypes

import concourse.bass as bass
import concourse.bacc as bacc
import concourse.tile as tile
from concourse import mybir
from concourse import bass_utils

F32 = mybir.dt.float32
BF16 = mybir.dt.bfloat16
NPBF16 = ml_dtypes.bfloat16

N = 50000
E = 800000
D = 256
C = 8
H = 64
GRP = 64
NPAD = 50176            # 784 groups of 64; divisible by 8*64
QROWS = NPAD // 8       # 6272 rows per dst eighth
QGRP = QROWS // GRP     # 98 groups per eighth
NPARTS = 4              # inter-layer AllGather parts
HR = QROWS // NPARTS    # 1568 rows: AG part size per rank
LO_ROWS = 25088         # = 8*2*HR: lo gather-table rows (mult of 64, < 32768)
LN_EPS = 1e-5
NCORES = 8
CH = 8                  # channels per core
BH = 12                 # chunks per stream batch
CALLB = 6               # chunks per dma_gather call (768 descs)
NSWQ = 4                # SWDGE queues; gather calls round-robin across them
REPLICA_GROUPS = [[0, 1, 2, 3, 4, 5, 6, 7]]


class Cfg:      # kept for test.py compatibility
    n_cores = NCORES


FULL = Cfg()


# ----------------------------------------------------------------------------
# Host-side preprocessing
# ----------------------------------------------------------------------------

LAST_META = None


def preprocess(cfg, x, edge_index, omega, proj, W1, W2, ln_gamma, ln_beta):
    src = np.asarray(edge_index[0], dtype=np.int64)
    dst = np.asarray(edge_index[1], dtype=np.int64)
    omega = np.asarray(omega, dtype=np.float32)
    x = np.asarray(x, dtype=np.float32)

    order = np.argsort(dst, kind="stable")
    src_s, dst_s, om_s = src[order], dst[order], omega[order]
    bounds = np.searchsorted(dst_s, [r * QROWS for r in range(NCORES + 1)])

    # per-core edge data in local coordinates
    cores = []
    for r in range(NCORES):
        b0, b1 = bounds[r], bounds[r + 1]
        s_r = src_s[b0:b1]
        d_r = dst_s[b0:b1] - r * QROWS
        jl = d_r // GRP
        cores.append({"src": s_r, "jl": jl, "col": d_r - jl * GRP,
                      "om": om_s[b0:b1],
                      "total": np.bincount(jl, minlength=QGRP)})

    # per-core group permutation: slot s hosts the s-th heaviest group
    perm = [np.argsort(-c["total"], kind="stable") for c in cores]
    inv_perm = []
    for r in range(NCORES):
        ip = np.empty(QGRP, np.int64)
        ip[perm[r]] = np.arange(QGRP)
        inv_perm.append(ip)
        cores[r]["slot"] = ip[cores[r]["jl"]]

    # h1 (slot/half-major) table row for each node-order padded index
    def h1_rows(n):
        r = n // QROWS
        q = n % QROWS
        jl = q // GRP
        w = q - jl * GRP
        qs = np.stack([inv_perm[rr] for rr in range(NCORES)])[r, jl] * GRP + w
        p = qs // HR
        return p * (NCORES * HR) + r * HR + (qs - p * HR)

    # per layer: table row of each edge's src
    layers = []     # layers[L][r][half] dicts
    for L in range(2):
        per_r = []
        for r in range(NCORES):
            c = cores[r]
            rows = c["src"] if L == 0 else h1_rows(c["src"])
            lo_mask = rows < LO_ROWS
            hs = []
            for m in (lo_mask, ~lo_mask):
                sel = np.nonzero(m)[0]
                # order within half by slot (stable)
                o = np.argsort(c["slot"][sel], kind="stable")
                sel = sel[o]
                slot = c["slot"][sel]
                hs.append({"rows": rows[sel], "slot": slot,
                           "col": c["col"][sel], "om": c["om"][sel],
                           "counts": np.bincount(slot, minlength=QGRP)})
            per_r.append(hs)
        layers.append(per_r)

    # SPMD-common chunk layout per layer/half (max over cores per slot)
    chunks = []  # chunks[L][half]
    for L in range(2):
        ch2 = []
        for h in range(2):
            cnt = np.stack([layers[L][r][h]["counts"] for r in range(NCORES)])
            ch2.append(((cnt.max(axis=0) + 127) // 128).astype(np.int64))
        tot0 = (ch2[0] + ch2[1]) == 0
        ch2[0] = np.maximum(ch2[0], tot0.astype(np.int64))
        chunks.append(ch2)

    # idx16 + omega/one-hot tiles for each (layer, half, core)
    idx16 = [[[None] * NCORES for _ in range(2)] for _ in range(2)]
    om_arr = [[[None] * NCORES for _ in range(2)] for _ in range(2)]
    oh_arr = [[[None] * NCORES for _ in range(2)] for _ in range(2)]
    for L in range(2):
        for h in range(2):
            padded = chunks[L][h] * 128
            starts = np.concatenate([[0], np.cumsum(padded)[:-1]])
            Ln = int(padded.sum())
            for r in range(NCORES):
                hd = layers[L][r][h]
                cum = np.concatenate([[0], np.cumsum(hd["counts"])[:-1]])
                rank = (np.arange(len(hd["rows"]))
                        - np.repeat(cum, hd["counts"]))
                pos = starts[hd["slot"]] + rank
                idx_vals = np.zeros(Ln, np.int64)
                idx_vals[pos] = hd["rows"] - (0 if h == 0 else LO_ROWS)
                assert idx_vals.min() >= 0 and idx_vals.max() < 32768
                idx16[L][h][r] = np.ascontiguousarray(
                    idx_vals.astype(np.int16).reshape(-1, 16).T)
                OM = np.zeros((Ln, CH), NPBF16)
                OM[pos] = hd["om"].astype(NPBF16)
                OH = np.zeros((Ln, GRP), NPBF16)
                OH[pos, hd["col"]] = 1.0
                om_arr[L][h][r] = np.ascontiguousarray(
                    OM.reshape(Ln // 128, 128, CH).transpose(1, 0, 2))
                oh_arr[L][h][r] = np.ascontiguousarray(
                    OH.reshape(Ln // 128, 128, GRP).transpose(1, 0, 2))

    xT = np.zeros((D, NPAD), NPBF16)
    xT[:, :N] = x.T.astype(NPBF16)

    skip_affine = bool(np.all(np.asarray(ln_gamma) == 1.0)
                       and np.all(np.asarray(ln_beta) == 0.0))

    proj_f = np.ascontiguousarray(
        np.asarray(proj, np.float32).reshape(D, C * H).astype(NPBF16))
    W1_f = np.ascontiguousarray(
        np.asarray(W1, np.float32).transpose(1, 0, 2).reshape(H, C * H)
        .astype(NPBF16))
    W2_f = np.ascontiguousarray(
        np.asarray(W2, np.float32).transpose(1, 0, 2).reshape(H, C * H)
        .astype(NPBF16))

    per_core = []
    for r in range(NCORES):
        m = {"xT": xT, "proj": proj_f, "W1": W1_f, "W2": W2_f,
             "idx_lo_a": idx16[0][0][r], "idx_hi_a": idx16[0][1][r],
             "idx_lo_b": idx16[1][0][r], "idx_hi_b": idx16[1][1][r],
             "om_lo_a": om_arr[0][0][r], "om_hi_a": om_arr[0][1][r],
             "om_lo_b": om_arr[1][0][r], "om_hi_b": om_arr[1][1][r],
             "oh_lo_a": oh_arr[0][0][r], "oh_hi_a": oh_arr[0][1][r],
             "oh_lo_b": oh_arr[1][0][r], "oh_hi_b": oh_arr[1][1][r]}
        if not skip_affine:
            m["gamma"] = np.asarray(ln_gamma, np.float32).reshape(1, H)
            m["beta"] = np.asarray(ln_beta, np.float32).reshape(1, H)
        per_core.append(m)

    meta = {
        "lo_a": [int(v) for v in chunks[0][0]],
        "hi_a": [int(v) for v in chunks[0][1]],
        "lo_b": [int(v) for v in chunks[1][0]],
        "hi_b": [int(v) for v in chunks[1][1]],
        "inv_perm": inv_perm,
    }
    global LAST_META
    LAST_META = meta
    return per_core, meta, None, skip_affine


# ----------------------------------------------------------------------------
# Bass program
# ----------------------------------------------------------------------------

QCTR = [0]   # global gather-call counter for SWDGE queue round-robin


class GatherStream:
    """Streams gather + omega + one-hot tiles for one (layer, half)."""

    def __init__(self, nc, name, idx_dram, om_dram, oh_dram, n_chunks,
                 table_view, gpool, spool, mpool, ipool):
        self.nc, self.name = nc, name
        self.idx_dram = idx_dram
        self.om_dram, self.oh_dram = om_dram, oh_dram
        self.n_chunks = n_chunks
        self.table_view = table_view
        self.gpool, self.spool, self.mpool, self.ipool = (
            gpool, spool, mpool, ipool)
        self.cur_bt = -1
        self.gtile = None
        self.otile = None
        self.mtile = None

    def _issue(self, bt):
        nc = self.nc
        b0 = bt * BH
        B = min(BH, self.n_chunks - b0)
        itile = self.ipool.tile([128, BH * 8], mybir.dt.int16, tag="i")
        idx_src = self.idx_dram[:, b0 * 8: b0 * 8 + B * 8]
        bcast = bass.AP(tensor=idx_src.tensor, offset=idx_src.offset,
                        ap=[[0, 8]] + idx_src.ap)
        nc.sync.dma_start(out=itile[:, :B * 8], in_=bcast)

        self.gtile = self.gpool.tile([128, BH, CH * H], BF16, tag="g")
        for cb in range(0, B, CALLB):
            nb = min(CALLB, B - cb)
            nc.gpsimd.dma_gather(
                out_ap=self.gtile[:, cb:cb + nb, :], in_ap=self.table_view,
                idxs_ap=itile[:, cb * 8:(cb + nb) * 8],
                num_idxs=nb * 128, num_idxs_reg=nb * 128,
                elem_size=CH * H, single_packet=False,
                queue_num=QCTR[0] % NSWQ)
            QCTR[0] += 1

        self.mtile = self.mpool.tile([128, BH, CH], BF16, tag="m")
        nc.sync.dma_start(out=self.mtile[:, :B, :],
                          in_=self.om_dram[:, b0:b0 + B, :])
        self.otile = self.spool.tile([128, BH, GRP], BF16, tag="s")
        nc.sync.dma_start(out=self.otile[:, :B, :],
                          in_=self.oh_dram[:, b0:b0 + B, :])
        self.cur_bt = bt

    def chunk(self, ci):
        bt, off = divmod(ci, BH)
        if bt != self.cur_bt:
            assert bt == self.cur_bt + 1
            self._issue(bt)
        return (self.gtile[:, off, :], self.mtile[:, off, :],
                self.otile[:, off, :])


def build_program(cfg, meta, _unused, skip_affine, num_devices=8):
    QCTR[0] = 0
    nc = bacc.Bacc("TRN2", target_bir_lowering=False, debug=False,
                   num_devices=num_devices,
                   num_swdge_queues=NSWQ,
                   dynamic_dma_scratch_size=32768)
    NLa, NHa = sum(meta["lo_a"]), sum(meta["hi_a"])
    NLb, NHb = sum(meta["lo_b"]), sum(meta["hi_b"])

    xT = nc.dram_tensor("xT", [D, NPAD], BF16, kind="ExternalInput").ap()
    proj = nc.dram_tensor("proj", [D, CH * H], BF16, kind="ExternalInput").ap()
    W1 = nc.dram_tensor("W1", [H, CH * H], BF16, kind="ExternalInput").ap()
    W2 = nc.dram_tensor("W2", [H, CH * H], BF16, kind="ExternalInput").ap()

    def decl(name, nchunks, dt2, shape_fn):
        return nc.dram_tensor(name, shape_fn(nchunks), dt2,
                              kind="ExternalInput").ap()

    idx_sh = lambda n: [16, n * 8]
    om_sh = lambda n: [128, n, CH]
    oh_sh = lambda n: [128, n, GRP]
    idx_lo_a = decl("idx_lo_a", NLa, mybir.dt.int16, idx_sh)
    idx_hi_a = decl("idx_hi_a", NHa, mybir.dt.int16, idx_sh)
    idx_lo_b = decl("idx_lo_b", NLb, mybir.dt.int16, idx_sh)
    idx_hi_b = decl("idx_hi_b", NHb, mybir.dt.int16, idx_sh)
    om_lo_a = decl("om_lo_a", NLa, BF16, om_sh)
    om_hi_a = decl("om_hi_a", NHa, BF16, om_sh)
    om_lo_b = decl("om_lo_b", NLb, BF16, om_sh)
    om_hi_b = decl("om_hi_b", NHb, BF16, om_sh)
    oh_lo_a = decl("oh_lo_a", NLa, BF16, oh_sh)
    oh_hi_a = decl("oh_hi_a", NHa, BF16, oh_sh)
    oh_lo_b = decl("oh_lo_b", NLb, BF16, oh_sh)
    oh_hi_b = decl("oh_hi_b", NHb, BF16, oh_sh)

    out = nc.dram_tensor("out", [QROWS, CH * H], F32,
                         kind="ExternalOutput").ap()
    if not skip_affine:
        gamma = nc.dram_tensor("gamma", [1, H], F32, kind="ExternalInput").ap()
        beta = nc.dram_tensor("beta", [1, H], F32, kind="ExternalInput").ap()

    import contextlib
    with tile.TileContext(nc) as tc:
        with contextlib.ExitStack() as _ctx:
            def _pool(**kw):
                return _ctx.enter_context(tc.tile_pool(**kw))
            dpool = _pool(name="dram", bufs=1, space="DRAM")
            singles = _pool(name="singles", bufs=1)
            xtpool = _pool(name="xt", bufs=3)
            pproj = _pool(name="pproj", bufs=2, space="PSUM")
            projsb = _pool(name="projsb", bufs=3)
            glo = _pool(name="glo", bufs=2)
            ghi = _pool(name="ghi", bufs=2)
            slo = _pool(name="slo", bufs=2)
            shi = _pool(name="shi", bufs=2)
            mlo = _pool(name="mlo", bufs=2)
            mhi = _pool(name="mhi", bufs=2)
            ilo = _pool(name="ilo", bufs=2)
            ihi = _pool(name="ihi", bufs=2)
            gspool = _pool(name="gs", bufs=4)
            paggT = _pool(name="paggT", bufs=2, space="PSUM")
            pout = _pool(name="pout", bufs=2, space="PSUM")
            convsb = _pool(name="convsb", bufs=3)
            obpool = _pool(name="ob", bufs=3)
            lnpool = _pool(name="ln", bufs=6)
            sqpool = _pool(name="sq", bufs=2)
            h0_lo = dpool.tile([LO_ROWS, CH * H], BF16)
            h0_hi = dpool.tile([NPAD - LO_ROWS, CH * H], BF16)
            h1_mine = dpool.tile([QROWS, CH * H], BF16)
            h1_full = dpool.tile([NPAD, CH * H], BF16)

            eps_t = singles.tile([128, 1], F32)
            nc.vector.memset(eps_t, LN_EPS)
            proj_t = singles.tile([128, D // 128, CH * H], BF16)
            nc.sync.dma_start(out=proj_t[:],
                              in_=proj.rearrange("(k p) h -> p k h", p=128))
            W1_t = singles.tile([H, CH * H], BF16)
            nc.sync.dma_start(out=W1_t[:], in_=W1[:])
            W2_t = singles.tile([H, CH * H], BF16)
            nc.sync.dma_start(out=W2_t[:], in_=W2[:])
            if not skip_affine:
                gamma_t = singles.tile([GRP, H], F32)
                nc.sync.dma_start(out=gamma_t[:], in_=bass.AP(
                    tensor=gamma.tensor, offset=gamma.offset,
                    ap=[[0, GRP]] + gamma.ap[1:]))
                beta_t = singles.tile([GRP, H], F32)
                nc.sync.dma_start(out=beta_t[:], in_=bass.AP(
                    tensor=beta.tensor, offset=beta.offset,
                    ap=[[0, GRP]] + beta.ap[1:]))

            # ---------------- phase A: h0 = x @ proj (full table) ----------
            # 2048-col batches; four 128-row result tiles packed into one
            # [128, 4, 512] SBUF tile and written with a single DMA per
            # 512 table rows (cuts the per-write HWDGE setup+sem chain 4x).
            KCH = D // 128
            COLB = 2048
            WPACK = 4
            ti = 0
            for c0 in range(0, NPAD, COLB):
                cb = min(COLB, NPAD - c0)
                xts = []
                for k in range(KCH):
                    xt_t = xtpool.tile([128, COLB], BF16, tag=f"xt{k}")
                    nc.sync.dma_start(out=xt_t[:, :cb],
                                      in_=xT[k * 128:(k + 1) * 128,
                                             c0:c0 + cb])
                    xts.append(xt_t)
                for w0 in range(0, cb, WPACK * 128):
                    wrows = min(WPACK * 128, cb - w0)
                    nw = wrows // 128
                    sb = projsb.tile([128, WPACK, CH * H], BF16, tag="sb")
                    for j in range(nw):
                        t0 = w0 + j * 128
                        ps = pproj.tile([128, CH * H], F32)
                        for k in range(KCH):
                            nc.tensor.matmul(out=ps[:],
                                             lhsT=xts[k][:, t0:t0 + 128],
                                             rhs=proj_t[:, k, :],
                                             start=(k == 0),
                                             stop=(k == KCH - 1))
                        if ti % 2 == 0:
                            nc.scalar.activation(
                                out=sb[:, j, :], in_=ps[:],
                                func=mybir.ActivationFunctionType.Copy)
                        else:
                            nc.vector.tensor_copy(out=sb[:, j, :], in_=ps[:])
                        ti += 1
                    r0 = c0 + w0
                    if r0 < LO_ROWS:
                        dst_rows = h0_lo[r0:r0 + wrows, :]
                    else:
                        dst_rows = h0_hi[r0 - LO_ROWS:r0 - LO_ROWS + wrows, :]
                    nc.sync.dma_start(
                        out=dst_rows.rearrange("(j p) f -> p j f", p=128),
                        in_=sb[:, :nw, :])
                    ti += 1

            # ---------------- conv layers ----------------
            NPAIR = CH // 2

            def conv_layer(lname, tlo, thi, idxlo, idxhi, omlo, omhi,
                           ohlo, ohhi, NLx, NHx,
                           lo_chunks, hi_chunks, W_t, dst, dst_dt, relu):
                s_lo = GatherStream(nc, f"lo{lname}", idxlo, omlo, ohlo, NLx,
                                    tlo, glo, slo, mlo, ilo)
                s_hi = GatherStream(nc, f"hi{lname}", idxhi, omhi, ohhi, NHx,
                                    thi, ghi, shi, mhi, ihi)
                ci_lo = ci_hi = 0
                for g in range(QGRP):
                    nlo, nhi = lo_chunks[g], hi_chunks[g]
                    total = nlo + nhi
                    assert total > 0
                    agg = paggT.tile([H, CH * GRP], F32, tag="agg")
                    done = 0
                    for st, nch, ci0 in ((s_lo, nlo, ci_lo),
                                         (s_hi, nhi, ci_hi)):
                        for j in range(nch):
                            g_ap, om_ap, oh_ap = st.chunk(ci0 + j)
                            # gs = g * omega (per-edge per-channel, bcast 64)
                            gs = gspool.tile([128, CH, H], BF16, tag="gs")
                            om_b = bass.AP(
                                tensor=om_ap.tensor, offset=om_ap.offset,
                                ap=[om_ap.ap[0], om_ap.ap[1], [0, H]])
                            nc.vector.tensor_tensor(
                                out=gs[:],
                                in0=g_ap.rearrange("p (c h) -> p c h", c=CH),
                                in1=om_b, op=mybir.AluOpType.mult)
                            gsf = gs[:].rearrange("p c h -> p (c h)")
                            for c in range(CH):
                                nc.tensor.matmul(
                                    out=agg[:, c * GRP:(c + 1) * GRP],
                                    lhsT=gsf[:, c * H:(c + 1) * H],
                                    rhs=oh_ap,
                                    start=(done == 0 and c == 0),
                                    stop=(done == total - 1 and c == CH - 1))
                            done += 1
                    ci_lo += nlo
                    ci_hi += nhi

                    aggsb = convsb.tile([H, CH * GRP], BF16, tag="aggsb")
                    if g % 2 == 0:
                        nc.scalar.activation(
                            out=aggsb[:], in_=agg[:],
                            func=mybir.ActivationFunctionType.Copy)
                    else:
                        nc.vector.tensor_copy(out=aggsb[:], in_=agg[:])
                    po = pout.tile([GRP, CH * H], F32, tag="po")
                    for c in range(CH):
                        nc.tensor.matmul(out=po[:, c * H:(c + 1) * H],
                                         lhsT=aggsb[:, c * GRP:(c + 1) * GRP],
                                         rhs=W_t[:, c * H:(c + 1) * H],
                                         start=(c == 0), stop=(c == CH - 1))
                    obrow = obpool.tile([GRP, CH * H], dst_dt, tag="obrow")
                    # Copy po to SBUF (DVE cannot read PSUM on both ports),
                    # then batched per-channel LN stats via strided reduces.
                    posb = sqpool.tile([GRP, CH * H], F32, tag="posb")
                    if g % 2 == 1:
                        nc.scalar.activation(
                            out=posb[:], in_=po[:],
                            func=mybir.ActivationFunctionType.Copy)
                    else:
                        nc.vector.tensor_copy(out=posb[:], in_=po[:])
                    po3 = posb[:].rearrange("p (c h) -> p c h", c=CH)
                    sum_t = lnpool.tile([GRP, CH], F32, tag="s")
                    sq = sqpool.tile([GRP, CH, H], F32, tag="sq")
                    sumsq = lnpool.tile([GRP, CH], F32, tag="ss")
                    s2 = lnpool.tile([GRP, CH], F32, tag="s2")
                    q_t = lnpool.tile([GRP, CH], F32, tag="q")
                    rstd = lnpool.tile([GRP, CH], F32, tag="rs")
                    nmr = lnpool.tile([GRP, CH], F32, tag="nm")
                    nc.vector.tensor_reduce(out=sum_t[:], in_=po3,
                                            axis=mybir.AxisListType.X,
                                            op=mybir.AluOpType.add)
                    nc.vector.tensor_mul(out=sq[:], in0=po3, in1=po3)
                    nc.vector.tensor_reduce(out=sumsq[:], in_=sq[:],
                                            axis=mybir.AxisListType.X,
                                            op=mybir.AluOpType.add)
                    nc.vector.tensor_mul(out=s2[:], in0=sum_t[:],
                                         in1=sum_t[:])
                    # q = sumsq - sum^2/H = H*var; rstd = 1/sqrt(q/H + eps)
                    nc.vector.scalar_tensor_tensor(
                        out=q_t[:], in0=s2[:], scalar=float(-1.0 / H),
                        in1=sumsq[:], op0=mybir.AluOpType.mult,
                        op1=mybir.AluOpType.add)
                    nc.scalar.activation(
                        out=rstd[:], in_=q_t[:],
                        func=mybir.ActivationFunctionType.Sqrt,
                        bias=eps_t[:GRP, :], scale=float(1.0 / H))
                    nc.vector.reciprocal(out=rstd[:], in_=rstd[:])
                    # nmr = -mean * rstd = (sum * -1/H) * rstd
                    nc.vector.scalar_tensor_tensor(
                        out=nmr[:], in0=sum_t[:], scalar=float(-1.0 / H),
                        in1=rstd[:], op0=mybir.AluOpType.mult,
                        op1=mybir.AluOpType.mult)
                    for c in range(CH):
                        poc = posb[:, c * H:(c + 1) * H]
                        if skip_affine:
                            nc.scalar.activation(
                                out=obrow[:, c * H:(c + 1) * H], in_=poc,
                                func=(mybir.ActivationFunctionType.Relu if relu
                                      else mybir.ActivationFunctionType.Identity),
                                bias=nmr[:, c:c + 1], scale=rstd[:, c:c + 1])
                        else:
                            ob = convsb.tile([GRP, H], F32, tag=f"eb{c}")
                            nc.scalar.activation(
                                out=ob[:], in_=poc,
                                func=mybir.ActivationFunctionType.Identity,
                                bias=nmr[:, c:c + 1], scale=rstd[:, c:c + 1])
                            nc.vector.tensor_mul(out=ob[:], in0=ob[:],
                                                 in1=gamma_t[:])
                            nc.vector.tensor_add(out=ob[:], in0=ob[:],
                                                 in1=beta_t[:])
                            if relu:
                                nc.vector.tensor_scalar_max(
                                    out=obrow[:, c * H:(c + 1) * H],
                                    in0=ob[:], scalar1=0.0)
                            else:
                                nc.vector.tensor_copy(
                                    out=obrow[:, c * H:(c + 1) * H], in_=ob[:])
                    nc.sync.dma_start(out=dst[g * GRP:(g + 1) * GRP, :],
                                      in_=obrow[:])

            conv_layer("a", h0_lo[:, :], h0_hi[:, :],
                       idx_lo_a, idx_hi_a, om_lo_a, om_hi_a,
                       oh_lo_a, oh_hi_a, NLa, NHa,
                       meta["lo_a"], meta["hi_a"], W1_t, h1_mine, BF16,
                       relu=True)
            # NPARTS contiguous AllGathers: part p gathers rows
            # [p*HR,(p+1)*HR) of every eighth into the part-major region of
            # h1_full; early parts overlap the tail of layer a.
            for p in range(NPARTS):
                nc.gpsimd.collective_compute(
                    "AllGather", mybir.AluOpType.bypass,
                    replica_groups=REPLICA_GROUPS,
                    ins=[h1_mine[p * HR:(p + 1) * HR, :].opt()],
                    outs=[h1_full[p * NCORES * HR:(p + 1) * NCORES * HR,
                                  :].opt()])
            conv_layer("b", h1_full[0:LO_ROWS, :], h1_full[LO_ROWS:NPAD, :],
                       idx_lo_b, idx_hi_b, om_lo_b, om_hi_b,
                       oh_lo_b, oh_hi_b,
                       NLb, NHb, meta["lo_b"], meta["hi_b"], W2_t, out, F32,
                       relu=False)

    nc.compile()
    return nc


# ----------------------------------------------------------------------------
# Entry point
# ----------------------------------------------------------------------------

def assemble(results, inv_perm=None):
    if inv_perm is None:
        inv_perm = LAST_META["inv_perm"]
    full = np.empty((N, C, H), np.float32)
    for k in range(NCORES):
        r0 = k * QROWS
        rows = min(QROWS, N - r0)
        if rows <= 0:
            continue
        o = results[k]["out"]                   # [QROWS, CH*H] f32 slot order
        q = np.arange(rows)
        jl = q // GRP
        w = q - jl * GRP
        slot_rows = inv_perm[k][jl] * GRP + w
        full[r0:r0 + rows, :, :] = o[slot_rows].reshape(rows, CH, H)
    return full


def kernel(x, edge_index, omega, proj, W1, W2, ln_gamma, ln_beta):
    per_core, meta, _, skip_affine = preprocess(
        FULL, x, edge_index, omega, proj, W1, W2, ln_gamma, ln_beta)
    nc = build_program(FULL, meta, None, skip_affine, num_devices=NCORES)
    res = bass_utils.run_bass_kernel_spmd(
        nc, per_core, core_ids=list(range(NCORES)))
    return np.ascontiguousarray(
        assemble(res.results, meta["inv_perm"]), dtype=np.float32)



# revision 35
# speedup vs baseline: 1.5199x; 1.0874x over previous
"""Trainium2 Bass kernel for nn_DisentangledGraphConvEncoder (octo v4).

Sharding: core k owns dst eighth k (6272 padded rows) and computes ALL 8
channels for those dsts. Tables are [NPAD, 512] bf16 (8 ch x 64 feats = 1KB
rows) so each gather descriptor moves 1KB.

v4 refinements over v3:
- Per-core groups are permuted by descending edge count before slot
  assignment, so the SPMD-common (max-over-cores) chunk layout wastes ~12%
  instead of ~24%.
- The h1 table uses a half-major layout (half p of every eighth, rank-major)
  so the inter-layer AllGather splits into two contiguous-output collectives;
  part 1 overlaps the tail of layer a.
- Layer a gathers from h0 in node order; layer b gathers from h1 in
  slot/half-major order — separate idx16 + S streams per layer.
"""

import numpy as np
import ml_dtypes

import concourse.bass as bass
import concourse.bacc as bacc
import concourse.tile as tile
from concourse import mybir
from concourse import bass_utils

F32 = mybir.dt.float32
BF16 = mybir.dt.bfloat16
NPBF16 = ml_dtypes.bfloat16

N = 50000
E = 800000
D = 256
C = 8
H = 64
GRP = 64
NPAD = 50176            # 784 groups of 64; divisible by 8*64
QROWS = NPAD // 8       # 6272 rows per dst eighth
QGRP = QROWS // GRP     # 98 groups per eighth
NPARTS = 4              # inter-layer AllGather parts
HR = QROWS // NPARTS    # 1568 rows: AG part size per rank
LO_ROWS = 25088         # = 8*2*HR: lo gather-table rows (mult of 64, < 32768)
LN_EPS = 1e-5
NCORES = 8
CH = 8                  # channels per core
BH = 12                 # chunks per stream batch
CALLB = 6               # chunks per dma_gather call (768 descs)
NSWQ = 4                # SWDGE queues; gather calls round-robin across them
REPLICA_GROUPS = [[0, 1, 2, 3, 4, 5, 6, 7]]


class Cfg:      # kept for test.py compatibility
    n_cores = NCORES


FULL = Cfg()


# ----------------------------------------------------------------------------
# Host-side preprocessing
# ----------------------------------------------------------------------------

LAST_META = None


def preprocess(cfg, x, edge_index, omega, proj, W1, W2, ln_gamma, ln_beta):
    src = np.asarray(edge_index[0], dtype=np.int64)
    dst = np.asarray(edge_index[1], dtype=np.int64)
    omega = np.asarray(omega, dtype=np.float32)
    x = np.asarray(x, dtype=np.float32)

    # Balanced assignment of global dst-groups to (core, slot): groups in
    # descending weight, each to the least-loaded core with open slots. Each
    # core thus receives its groups heaviest-first (slot s = s-th heaviest),
    # which both balances per-core edge totals and minimizes the SPMD
    # max-over-cores chunk layout waste.
    NGRP = NPAD // GRP
    g_of = dst // GRP
    weight = np.bincount(g_of, minlength=NGRP)
    core_of_group = np.empty(NGRP, np.int64)
    slot_of_group = np.empty(NGRP, np.int64)
    totals = np.zeros(NCORES, np.int64)
    counts = np.zeros(NCORES, np.int64)
    for g in np.argsort(-weight, kind="stable"):
        elig = np.nonzero(counts < QGRP)[0]
        k = elig[np.argmin(totals[elig])]
        core_of_group[g] = k
        slot_of_group[g] = counts[k]
        totals[k] += weight[g]
        counts[k] += 1

    core_of_edge = core_of_group[g_of]
    cores = []
    for r in range(NCORES):
        m = core_of_edge == r
        cores.append({"src": src[m], "slot": slot_of_group[g_of[m]],
                      "col": dst[m] % GRP, "om": omega[m]})

    # h1 (slot/part-major) table row for each node index
    def h1_rows(n):
        g = n // GRP
        w = n % GRP
        r = core_of_group[g]
        qs = slot_of_group[g] * GRP + w
        p = qs // HR
        return p * (NCORES * HR) + r * HR + (qs - p * HR)

    # per layer: table row of each edge's src
    layers = []     # layers[L][r][half] dicts
    for L in range(2):
        per_r = []
        for r in range(NCORES):
            c = cores[r]
            rows = c["src"] if L == 0 else h1_rows(c["src"])
            lo_mask = rows < LO_ROWS
            hs = []
            for m in (lo_mask, ~lo_mask):
                sel = np.nonzero(m)[0]
                # order within half by slot (stable)
                o = np.argsort(c["slot"][sel], kind="stable")
                sel = sel[o]
                slot = c["slot"][sel]
                hs.append({"rows": rows[sel], "slot": slot,
                           "col": c["col"][sel], "om": c["om"][sel],
                           "counts": np.bincount(slot, minlength=QGRP)})
            per_r.append(hs)
        layers.append(per_r)

    # SPMD-common chunk layout per layer/half (max over cores per slot)
    chunks = []  # chunks[L][half]
    for L in range(2):
        ch2 = []
        for h in range(2):
            cnt = np.stack([layers[L][r][h]["counts"] for r in range(NCORES)])
            ch2.append(((cnt.max(axis=0) + 127) // 128).astype(np.int64))
        tot0 = (ch2[0] + ch2[1]) == 0
        ch2[0] = np.maximum(ch2[0], tot0.astype(np.int64))
        chunks.append(ch2)

    # idx16 + omega/one-hot tiles for each (layer, half, core)
    idx16 = [[[None] * NCORES for _ in range(2)] for _ in range(2)]
    om_arr = [[[None] * NCORES for _ in range(2)] for _ in range(2)]
    oh_arr = [[[None] * NCORES for _ in range(2)] for _ in range(2)]
    for L in range(2):
        for h in range(2):
            padded = chunks[L][h] * 128
            starts = np.concatenate([[0], np.cumsum(padded)[:-1]])
            Ln = int(padded.sum())
            for r in range(NCORES):
                hd = layers[L][r][h]
                cum = np.concatenate([[0], np.cumsum(hd["counts"])[:-1]])
                rank = (np.arange(len(hd["rows"]))
                        - np.repeat(cum, hd["counts"]))
                pos = starts[hd["slot"]] + rank
                idx_vals = np.zeros(Ln, np.int64)
                idx_vals[pos] = hd["rows"] - (0 if h == 0 else LO_ROWS)
                assert idx_vals.min() >= 0 and idx_vals.max() < 32768
                idx16[L][h][r] = np.ascontiguousarray(
                    idx_vals.astype(np.int16).reshape(-1, 16).T)
                OM = np.zeros((Ln, CH), NPBF16)
                OM[pos] = hd["om"].astype(NPBF16)
                OH = np.zeros((Ln, GRP), NPBF16)
                OH[pos, hd["col"]] = 1.0
                om_arr[L][h][r] = np.ascontiguousarray(
                    OM.reshape(Ln // 128, 128, CH).transpose(1, 0, 2))
                oh_arr[L][h][r] = np.ascontiguousarray(
                    OH.reshape(Ln // 128, 128, GRP).transpose(1, 0, 2))

    xT = np.zeros((D, NPAD), NPBF16)
    xT[:, :N] = x.T.astype(NPBF16)

    skip_affine = bool(np.all(np.asarray(ln_gamma) == 1.0)
                       and np.all(np.asarray(ln_beta) == 0.0))

    proj_f = np.ascontiguousarray(
        np.asarray(proj, np.float32).reshape(D, C * H).astype(NPBF16))
    W1_f = np.ascontiguousarray(
        np.asarray(W1, np.float32).transpose(1, 0, 2).reshape(H, C * H)
        .astype(NPBF16))
    W2_f = np.ascontiguousarray(
        np.asarray(W2, np.float32).transpose(1, 0, 2).reshape(H, C * H)
        .astype(NPBF16))

    per_core = []
    for r in range(NCORES):
        m = {"xT": xT, "proj": proj_f, "W1": W1_f, "W2": W2_f,
             "idx_lo_a": idx16[0][0][r], "idx_hi_a": idx16[0][1][r],
             "idx_lo_b": idx16[1][0][r], "idx_hi_b": idx16[1][1][r],
             "om_lo_a": om_arr[0][0][r], "om_hi_a": om_arr[0][1][r],
             "om_lo_b": om_arr[1][0][r], "om_hi_b": om_arr[1][1][r],
             "oh_lo_a": oh_arr[0][0][r], "oh_hi_a": oh_arr[0][1][r],
             "oh_lo_b": oh_arr[1][0][r], "oh_hi_b": oh_arr[1][1][r]}
        if not skip_affine:
            m["gamma"] = np.asarray(ln_gamma, np.float32).reshape(1, H)
            m["beta"] = np.asarray(ln_beta, np.float32).reshape(1, H)
        per_core.append(m)

    meta = {
        "lo_a": [int(v) for v in chunks[0][0]],
        "hi_a": [int(v) for v in chunks[0][1]],
        "lo_b": [int(v) for v in chunks[1][0]],
        "hi_b": [int(v) for v in chunks[1][1]],
        "core_of_group": core_of_group,
        "slot_of_group": slot_of_group,
    }
    global LAST_META
    LAST_META = meta
    return per_core, meta, None, skip_affine


# ----------------------------------------------------------------------------
# Bass program
# ----------------------------------------------------------------------------

QCTR = [0]   # global gather-call counter for SWDGE queue round-robin


class GatherStream:
    """Streams gather + omega + one-hot tiles for one (layer, half)."""

    def __init__(self, nc, name, idx_dram, om_dram, oh_dram, n_chunks,
                 table_view, gpool, spool, mpool, ipool):
        self.nc, self.name = nc, name
        self.idx_dram = idx_dram
        self.om_dram, self.oh_dram = om_dram, oh_dram
        self.n_chunks = n_chunks
        self.table_view = table_view
        self.gpool, self.spool, self.mpool, self.ipool = (
            gpool, spool, mpool, ipool)
        self.cur_bt = -1
        self.gtile = None
        self.otile = None
        self.mtile = None

    def _issue(self, bt):
        nc = self.nc
        b0 = bt * BH
        B = min(BH, self.n_chunks - b0)
        itile = self.ipool.tile([128, BH * 8], mybir.dt.int16, tag="i")
        idx_src = self.idx_dram[:, b0 * 8: b0 * 8 + B * 8]
        bcast = bass.AP(tensor=idx_src.tensor, offset=idx_src.offset,
                        ap=[[0, 8]] + idx_src.ap)
        nc.sync.dma_start(out=itile[:, :B * 8], in_=bcast)

        self.gtile = self.gpool.tile([128, BH, CH * H], BF16, tag="g")
        for cb in range(0, B, CALLB):
            nb = min(CALLB, B - cb)
            nc.gpsimd.dma_gather(
                out_ap=self.gtile[:, cb:cb + nb, :], in_ap=self.table_view,
                idxs_ap=itile[:, cb * 8:(cb + nb) * 8],
                num_idxs=nb * 128, num_idxs_reg=nb * 128,
                elem_size=CH * H, single_packet=False,
                queue_num=QCTR[0] % NSWQ)
            QCTR[0] += 1

        self.mtile = self.mpool.tile([128, BH, CH], BF16, tag="m")
        nc.sync.dma_start(out=self.mtile[:, :B, :],
                          in_=self.om_dram[:, b0:b0 + B, :])
        self.otile = self.spool.tile([128, BH, GRP], BF16, tag="s")
        nc.sync.dma_start(out=self.otile[:, :B, :],
                          in_=self.oh_dram[:, b0:b0 + B, :])
        self.cur_bt = bt

    def chunk(self, ci):
        bt, off = divmod(ci, BH)
        if bt != self.cur_bt:
            assert bt == self.cur_bt + 1
            self._issue(bt)
        return (self.gtile[:, off, :], self.mtile[:, off, :],
                self.otile[:, off, :])


def build_program(cfg, meta, _unused, skip_affine, num_devices=8):
    QCTR[0] = 0
    nc = bacc.Bacc("TRN2", target_bir_lowering=False, debug=False,
                   num_devices=num_devices,
                   num_swdge_queues=NSWQ,
                   dynamic_dma_scratch_size=32768)
    NLa, NHa = sum(meta["lo_a"]), sum(meta["hi_a"])
    NLb, NHb = sum(meta["lo_b"]), sum(meta["hi_b"])

    xT = nc.dram_tensor("xT", [D, NPAD], BF16, kind="ExternalInput").ap()
    proj = nc.dram_tensor("proj", [D, CH * H], BF16, kind="ExternalInput").ap()
    W1 = nc.dram_tensor("W1", [H, CH * H], BF16, kind="ExternalInput").ap()
    W2 = nc.dram_tensor("W2", [H, CH * H], BF16, kind="ExternalInput").ap()

    def decl(name, nchunks, dt2, shape_fn):
        return nc.dram_tensor(name, shape_fn(nchunks), dt2,
                              kind="ExternalInput").ap()

    idx_sh = lambda n: [16, n * 8]
    om_sh = lambda n: [128, n, CH]
    oh_sh = lambda n: [128, n, GRP]
    idx_lo_a = decl("idx_lo_a", NLa, mybir.dt.int16, idx_sh)
    idx_hi_a = decl("idx_hi_a", NHa, mybir.dt.int16, idx_sh)
    idx_lo_b = decl("idx_lo_b", NLb, mybir.dt.int16, idx_sh)
    idx_hi_b = decl("idx_hi_b", NHb, mybir.dt.int16, idx_sh)
    om_lo_a = decl("om_lo_a", NLa, BF16, om_sh)
    om_hi_a = decl("om_hi_a", NHa, BF16, om_sh)
    om_lo_b = decl("om_lo_b", NLb, BF16, om_sh)
    om_hi_b = decl("om_hi_b", NHb, BF16, om_sh)
    oh_lo_a = decl("oh_lo_a", NLa, BF16, oh_sh)
    oh_hi_a = decl("oh_hi_a", NHa, BF16, oh_sh)
    oh_lo_b = decl("oh_lo_b", NLb, BF16, oh_sh)
    oh_hi_b = decl("oh_hi_b", NHb, BF16, oh_sh)

    out = nc.dram_tensor("out", [QROWS, CH * H], F32,
                         kind="ExternalOutput").ap()
    if not skip_affine:
        gamma = nc.dram_tensor("gamma", [1, H], F32, kind="ExternalInput").ap()
        beta = nc.dram_tensor("beta", [1, H], F32, kind="ExternalInput").ap()

    import contextlib
    with tile.TileContext(nc) as tc:
        with contextlib.ExitStack() as _ctx:
            def _pool(**kw):
                return _ctx.enter_context(tc.tile_pool(**kw))
            dpool = _pool(name="dram", bufs=1, space="DRAM")
            singles = _pool(name="singles", bufs=1)
            xtpool = _pool(name="xt", bufs=3)
            pproj = _pool(name="pproj", bufs=2, space="PSUM")
            projsb = _pool(name="projsb", bufs=3)
            glo = _pool(name="glo", bufs=2)
            ghi = _pool(name="ghi", bufs=2)
            slo = _pool(name="slo", bufs=2)
            shi = _pool(name="shi", bufs=2)
            mlo = _pool(name="mlo", bufs=2)
            mhi = _pool(name="mhi", bufs=2)
            ilo = _pool(name="ilo", bufs=2)
            ihi = _pool(name="ihi", bufs=2)
            gspool = _pool(name="gs", bufs=3)
            paggT = _pool(name="paggT", bufs=2, space="PSUM")
            pout = _pool(name="pout", bufs=2, space="PSUM")
            convsb = _pool(name="convsb", bufs=3)
            obpool = _pool(name="ob", bufs=3)
            lnpool = _pool(name="ln", bufs=6)
            sqpool = _pool(name="sq", bufs=2)
            h0_lo = dpool.tile([LO_ROWS, CH * H], BF16)
            h0_hi = dpool.tile([NPAD - LO_ROWS, CH * H], BF16)
            h1_mine = dpool.tile([QROWS, CH * H], BF16)
            h1_full = nc.dram_tensor("h1_full", [NPAD, CH * H], BF16,
                                     kind="Internal",
                                     addr_space="Shared").ap()

            eps_t = singles.tile([128, 1], F32)
            nc.vector.memset(eps_t, LN_EPS)
            proj_t = singles.tile([128, D // 128, CH * H], BF16)
            nc.sync.dma_start(out=proj_t[:],
                              in_=proj.rearrange("(k p) h -> p k h", p=128))
            W1_t = singles.tile([H, CH * H], BF16)
            nc.sync.dma_start(out=W1_t[:], in_=W1[:])
            W2_t = singles.tile([H, CH * H], BF16)
            nc.sync.dma_start(out=W2_t[:], in_=W2[:])
            if not skip_affine:
                gamma_t = singles.tile([GRP, H], F32)
                nc.sync.dma_start(out=gamma_t[:], in_=bass.AP(
                    tensor=gamma.tensor, offset=gamma.offset,
                    ap=[[0, GRP]] + gamma.ap[1:]))
                beta_t = singles.tile([GRP, H], F32)
                nc.sync.dma_start(out=beta_t[:], in_=bass.AP(
                    tensor=beta.tensor, offset=beta.offset,
                    ap=[[0, GRP]] + beta.ap[1:]))

            # ---------------- phase A: h0 = x @ proj (full table) ----------
            # 2048-col batches; four 128-row result tiles packed into one
            # [128, 4, 512] SBUF tile and written with a single DMA per
            # 512 table rows (cuts the per-write HWDGE setup+sem chain 4x).
            KCH = D // 128
            COLB = 2048
            WPACK = 4
            ti = 0
            for c0 in range(0, NPAD, COLB):
                cb = min(COLB, NPAD - c0)
                xts = []
                for k in range(KCH):
                    xt_t = xtpool.tile([128, COLB], BF16, tag=f"xt{k}")
                    nc.sync.dma_start(out=xt_t[:, :cb],
                                      in_=xT[k * 128:(k + 1) * 128,
                                             c0:c0 + cb])
                    xts.append(xt_t)
                for w0 in range(0, cb, WPACK * 128):
                    wrows = min(WPACK * 128, cb - w0)
                    nw = wrows // 128
                    sb = projsb.tile([128, WPACK, CH * H], BF16, tag="sb")
                    for j in range(nw):
                        t0 = w0 + j * 128
                        ps = pproj.tile([128, CH * H], F32)
                        for k in range(KCH):
                            nc.tensor.matmul(out=ps[:],
                                             lhsT=xts[k][:, t0:t0 + 128],
                                             rhs=proj_t[:, k, :],
                                             start=(k == 0),
                                             stop=(k == KCH - 1))
                        if ti % 2 == 0:
                            nc.scalar.activation(
                                out=sb[:, j, :], in_=ps[:],
                                func=mybir.ActivationFunctionType.Copy)
                        else:
                            nc.vector.tensor_copy(out=sb[:, j, :], in_=ps[:])
                        ti += 1
                    r0 = c0 + w0
                    if r0 < LO_ROWS:
                        dst_rows = h0_lo[r0:r0 + wrows, :]
                    else:
                        dst_rows = h0_hi[r0 - LO_ROWS:r0 - LO_ROWS + wrows, :]
                    nc.sync.dma_start(
                        out=dst_rows.rearrange("(j p) f -> p j f", p=128),
                        in_=sb[:, :nw, :])
                    ti += 1

            # ---------------- conv layers ----------------
            NPAIR = CH // 2

            def conv_layer(lname, tlo, thi, idxlo, idxhi, omlo, omhi,
                           ohlo, ohhi, NLx, NHx,
                           lo_chunks, hi_chunks, W_t, dst, dst_dt, relu):
                s_lo = GatherStream(nc, f"lo{lname}", idxlo, omlo, ohlo, NLx,
                                    tlo, glo, slo, mlo, ilo)
                s_hi = GatherStream(nc, f"hi{lname}", idxhi, omhi, ohhi, NHx,
                                    thi, ghi, shi, mhi, ihi)
                ci_lo = ci_hi = 0
                for g in range(QGRP):
                    nlo, nhi = lo_chunks[g], hi_chunks[g]
                    total = nlo + nhi
                    assert total > 0
                    agg = paggT.tile([H, CH * GRP], F32, tag="agg")
                    done = 0
                    for st, nch, ci0 in ((s_lo, nlo, ci_lo),
                                         (s_hi, nhi, ci_hi)):
                        j = 0
                        while j < nch:
                            ci = ci0 + j
                            bt, off = divmod(ci, BH)
                            if bt != st.cur_bt:
                                assert bt == st.cur_bt + 1
                                st._issue(bt)
                            n = min(4, nch - j, BH - off)
                            # gs = g * omega for n chunks in one DVE op
                            gs = gspool.tile([128, 4, CH, H], BF16, tag="gs")
                            om_ap = st.mtile[:, off:off + n, :]
                            om_b = bass.AP(
                                tensor=om_ap.tensor, offset=om_ap.offset,
                                ap=[om_ap.ap[0], om_ap.ap[1], om_ap.ap[2],
                                    [0, H]])
                            nc.vector.tensor_tensor(
                                out=gs[:, :n, :, :],
                                in0=st.gtile[:, off:off + n, :].rearrange(
                                    "p n (c h) -> p n c h", c=CH),
                                in1=om_b, op=mybir.AluOpType.mult)
                            for k in range(n):
                                gsf = gs[:, k, :, :].rearrange(
                                    "p c h -> p (c h)")
                                oh_ap = st.otile[:, off + k, :]
                                for c in range(CH):
                                    nc.tensor.matmul(
                                        out=agg[:, c * GRP:(c + 1) * GRP],
                                        lhsT=gsf[:, c * H:(c + 1) * H],
                                        rhs=oh_ap,
                                        start=(done == 0 and c == 0),
                                        stop=(done == total - 1
                                              and c == CH - 1))
                                done += 1
                            j += n
                    ci_lo += nlo
                    ci_hi += nhi

                    aggsb = convsb.tile([H, CH * GRP], BF16, tag="aggsb")
                    if g % 2 == 0:
                        nc.scalar.activation(
                            out=aggsb[:], in_=agg[:],
                            func=mybir.ActivationFunctionType.Copy)
                    else:
                        nc.vector.tensor_copy(out=aggsb[:], in_=agg[:])
                    po = pout.tile([GRP, CH * H], F32, tag="po")
                    for c in range(CH):
                        nc.tensor.matmul(out=po[:, c * H:(c + 1) * H],
                                         lhsT=aggsb[:, c * GRP:(c + 1) * GRP],
                                         rhs=W_t[:, c * H:(c + 1) * H],
                                         start=(c == 0), stop=(c == CH - 1))
                    obrow = obpool.tile([GRP, CH * H], dst_dt, tag="obrow")
                    # Copy po to SBUF (DVE cannot read PSUM on both ports),
                    # then batched per-channel LN stats via strided reduces.
                    posb = sqpool.tile([GRP, CH * H], F32, tag="posb")
                    if g % 2 == 1:
                        nc.scalar.activation(
                            out=posb[:], in_=po[:],
                            func=mybir.ActivationFunctionType.Copy)
                    else:
                        nc.vector.tensor_copy(out=posb[:], in_=po[:])
                    po3 = posb[:].rearrange("p (c h) -> p c h", c=CH)
                    sum_t = lnpool.tile([GRP, CH], F32, tag="s")
                    sq = sqpool.tile([GRP, CH, H], F32, tag="sq")
                    sumsq = lnpool.tile([GRP, CH], F32, tag="ss")
                    s2 = lnpool.tile([GRP, CH], F32, tag="s2")
                    q_t = lnpool.tile([GRP, CH], F32, tag="q")
                    rstd = lnpool.tile([GRP, CH], F32, tag="rs")
                    nmr = lnpool.tile([GRP, CH], F32, tag="nm")
                    nc.vector.tensor_reduce(out=sum_t[:], in_=po3,
                                            axis=mybir.AxisListType.X,
                                            op=mybir.AluOpType.add)
                    nc.vector.tensor_mul(out=sq[:], in0=po3, in1=po3)
                    nc.vector.tensor_reduce(out=sumsq[:], in_=sq[:],
                                            axis=mybir.AxisListType.X,
                                            op=mybir.AluOpType.add)
                    nc.vector.tensor_mul(out=s2[:], in0=sum_t[:],
                                         in1=sum_t[:])
                    # q = sumsq - sum^2/H = H*var; rstd = 1/sqrt(q/H + eps)
                    nc.vector.scalar_tensor_tensor(
                        out=q_t[:], in0=s2[:], scalar=float(-1.0 / H),
                        in1=sumsq[:], op0=mybir.AluOpType.mult,
                        op1=mybir.AluOpType.add)
                    nc.scalar.activation(
                        out=rstd[:], in_=q_t[:],
                        func=mybir.ActivationFunctionType.Sqrt,
                        bias=eps_t[:GRP, :], scale=float(1.0 / H))
                    nc.vector.reciprocal(out=rstd[:], in_=rstd[:])
                    # nmr = -mean * rstd = (sum * -1/H) * rstd
                    nc.vector.scalar_tensor_tensor(
                        out=nmr[:], in0=sum_t[:], scalar=float(-1.0 / H),
                        in1=rstd[:], op0=mybir.AluOpType.mult,
                        op1=mybir.AluOpType.mult)
                    for c in range(CH):
                        poc = posb[:, c * H:(c + 1) * H]
                        if skip_affine:
                            nc.scalar.activation(
                                out=obrow[:, c * H:(c + 1) * H], in_=poc,
                                func=(mybir.ActivationFunctionType.Relu if relu
                                      else mybir.ActivationFunctionType.Identity),
                                bias=nmr[:, c:c + 1], scale=rstd[:, c:c + 1])
                        else:
                            ob = convsb.tile([GRP, H], F32, tag=f"eb{c}")
                            nc.scalar.activation(
                                out=ob[:], in_=poc,
                                func=mybir.ActivationFunctionType.Identity,
                                bias=nmr[:, c:c + 1], scale=rstd[:, c:c + 1])
                            nc.vector.tensor_mul(out=ob[:], in0=ob[:],
                                                 in1=gamma_t[:])
                            nc.vector.tensor_add(out=ob[:], in0=ob[:],
                                                 in1=beta_t[:])
                            if relu:
                                nc.vector.tensor_scalar_max(
                                    out=obrow[:, c * H:(c + 1) * H],
                                    in0=ob[:], scalar1=0.0)
                            else:
                                nc.vector.tensor_copy(
                                    out=obrow[:, c * H:(c + 1) * H], in_=ob[:])
                    nc.sync.dma_start(out=dst[g * GRP:(g + 1) * GRP, :],
                                      in_=obrow[:])

            conv_layer("a", h0_lo[:, :], h0_hi[:, :],
                       idx_lo_a, idx_hi_a, om_lo_a, om_hi_a,
                       oh_lo_a, oh_hi_a, NLa, NHa,
                       meta["lo_a"], meta["hi_a"], W1_t, h1_mine, BF16,
                       relu=True)
            # NPARTS contiguous AllGathers: part p gathers rows
            # [p*HR,(p+1)*HR) of every eighth into the part-major region of
            # h1_full; early parts overlap the tail of layer a.
            for p in range(NPARTS):
                nc.gpsimd.collective_compute(
                    "AllGather", mybir.AluOpType.bypass,
                    replica_groups=REPLICA_GROUPS,
                    ins=[h1_mine[p * HR:(p + 1) * HR, :].opt()],
                    outs=[h1_full[p * NCORES * HR:(p + 1) * NCORES * HR,
                                  :].opt()])
            conv_layer("b", h1_full[0:LO_ROWS, :], h1_full[LO_ROWS:NPAD, :],
                       idx_lo_b, idx_hi_b, om_lo_b, om_hi_b,
                       oh_lo_b, oh_hi_b,
                       NLb, NHb, meta["lo_b"], meta["hi_b"], W2_t, out, F32,
                       relu=False)

    nc.compile()
    return nc


# ----------------------------------------------------------------------------
# Entry point
# ----------------------------------------------------------------------------

def assemble(results, meta=None):
    if meta is None:
        meta = LAST_META
    cog = np.asarray(meta["core_of_group"])
    sog = np.asarray(meta["slot_of_group"])
    full = np.empty((N, C, H), np.float32)
    n = np.arange(N)
    g = n // GRP
    w = n - g * GRP
    r_of = cog[g]
    rows = sog[g] * GRP + w
    for k in range(NCORES):
        m = r_of == k
        full[m] = results[k]["out"][rows[m]].reshape(-1, C, H)
    return full


def kernel(x, edge_index, omega, proj, W1, W2, ln_gamma, ln_beta):
    per_core, meta, _, skip_affine = preprocess(
        FULL, x, edge_index, omega, proj, W1, W2, ln_gamma, ln_beta)
    nc = build_program(FULL, meta, None, skip_affine, num_devices=NCORES)
    res = bass_utils.run_bass_kernel_spmd(
        nc, per_core, core_ids=list(range(NCORES)))
    return np.ascontiguousarray(
        assemble(res.results, meta), dtype=np.float32)

